# revision 31
# baseline (speedup 1.0000x reference)
"""Trainium2 Bass kernel for a dense transformer block (B=8, T=1024, C=1024, H=16, FF=4096).

Sharding: data-parallel over batch - one batch element per NeuronCore (8 cores),
no collectives.

Speed levers vs the bf16 baseline (631 us):
  * fp8e4m3 DoubleRow matmuls for QKV, proj, FFN1, FFN2 and the attention AV
    contraction: each PE instruction contracts 256 rows instead of 128 at the
    same column rate, halving PE time for the big GEMMs. The fake-quantized
    weights are integers k*2^-8 with |k|<=8 - EXACT in fp8e4m3; only
    activations pick up ~2% rounding noise (end-to-end rel err ~9.6e-3,
    gate 2e-2).
  * Engine balance: during attention ACT runs only Exp (no act-table swaps);
    DVE does LN stats, transpose evacuation, softmax normalize, residual
    adds; GpSimd does causal masks, zero-fills and r broadcasts.
  * Pipelining: weight DMAs split into chunks across queues; QKV starts on
    the first t-half of LN1; S/exp and AV interleave per s-tile pair; LN2
    transposes stagger between proj matmuls; FFN1 runs half-major so FFN2
    chases it; outputs stream out per t-group.

Layouts (per core):
  hT    [128, 8, 512] fp8 x2 : hT[p, i, t] = h[t, 128*i + p] (t-halves; LN1 then LN2)
  w*8   [128, K/128, M] fp8  : w[p, i, m] = W[m, 128*i + p] (DoubleRow pairs = dim1)
  qT/kT [128, 1024] fp8 per head-pair (rows 64e+d)
  v8    [128, 8, 16, 128] fp8: cols 0-63 ones (sums -> avp rows 0-63), 64-127 = v
  exp8  [128, 8, 2, 1024] fp8 per pair: exp8[s, j, e, t] = exp(S[t, 128j+s]) unnorm
  attT8 [128, 8, 1024] fp8   : attT8[64e+d, p8, t] = normalized att
  f1    [128, 32, 512] fp8 x2: f1[p, f, t] = relu(ffn1)[t, 128f+p] (t-halves)
"""

import os
import numpy as np
import ml_dtypes

DEBUG = bool(int(os.environ.get("BASSDBG", "0")))

B, T, C, H = 8, 1024, 1024, 16
HS = C // H          # 64
FF = 4 * C           # 4096
EPS = 1e-5
NT = T // 128        # 8 t-tiles
NCI = C // 128       # 8 c-tiles
NFF = FF // 128      # 32 ff-tiles
VW = 128             # per-head stride in v8: cols 0-63 ones (sums on avp row 0
                     # for partition_broadcast), v in cols 64-127 (partition
                     # ranges of 64 must start at 0 or 64)
SM_SCALE = 1.0 / 32.0  # C ** -0.5

_CACHE = {}

npf8 = ml_dtypes.float8_e4m3  # TRN fp8e4 (max 240)


# ----------------------------------------------------------------------------
# host-side math (exact reference semantics)
# ----------------------------------------------------------------------------

def _quant_weight(W, e, b):
    W = np.asarray(W, np.float32)
    e = np.asarray(e, np.float32)
    b = np.asarray(b, np.float32)
    b_rel = np.maximum(b, 0.0)
    mn = np.where(b_rel > 0, -(2.0 ** (b_rel - 1)), 0.0)
    mx = np.where(b_rel > 0, 2.0 ** (b_rel - 1) - 1.0, 0.0)
    qw = np.clip((2.0 ** (-e)) * W, mn, mx)
    w = np.round(qw)  # round-half-even, same as jnp.round
    return ((2.0 ** e) * w).astype(np.float32)


def _dr_layout(WT):
    """[K, M] -> [128, K//128, M] fp8 DoubleRow layout: out[p, i, m] = WT[128i+p, m]."""
    K, M = WT.shape
    return np.ascontiguousarray(
        WT.reshape(K // 128, 128, M).transpose(1, 0, 2)).astype(npf8)


def _prep(inputs):
    f32 = np.float32
    g1 = np.asarray(inputs["g1"], f32)
    be1 = np.asarray(inputs["be1"], f32)
    g2 = np.asarray(inputs["g2"], f32)
    be2 = np.asarray(inputs["be2"], f32)

    Wq = _quant_weight(inputs["Wq"], inputs["eq"], inputs["bq"])  # [H,HS,C]
    Wk = _quant_weight(inputs["Wk"], inputs["ek"], inputs["bk"])
    Wv = _quant_weight(inputs["Wv"], inputs["ev"], inputs["bv"])
    Wp = _quant_weight(inputs["Wp"], inputs["ep"], inputs["bp"])  # [C,C]
    W1 = _quant_weight(inputs["W1"], inputs["e1"], inputs["b1"])  # [FF,C]
    W2 = _quant_weight(inputs["W2"], inputs["e2"], inputs["b2"])  # [C,FF]

    # fold LN gains into the adjacent weights (identity when g == 1, so the
    # fp8 cast of the quantized weights stays exact in that case)
    Wqf = (Wq * g1[None, None, :]).reshape(H * HS, C)
    Wkf = (Wk * g1[None, None, :]).reshape(H * HS, C)
    Wvf = (Wv * g1[None, None, :]).reshape(H * HS, C)
    W1f = W1 * g2[None, :]

    d = {
        "wq8": _dr_layout(Wqf.T), "wk8": _dr_layout(Wkf.T), "wv8": _dr_layout(Wvf.T),
        "wp8": _dr_layout(np.ascontiguousarray(Wp.T)),
        "w18": _dr_layout(np.ascontiguousarray(W1f.T)),
        "w28": _dr_layout(np.ascontiguousarray(W2.T)),
    }
    # biases from LN betas routed through the projections
    qb = (Wqf @ be1).astype(f32)   # [H*HS]
    kb = (Wkf @ be1).astype(f32)
    vb = (Wvf @ be1).astype(f32)
    b1e = (np.asarray(inputs["bias1"], f32) + W1 @ be2).astype(f32)  # [FF]
    d["qb"] = np.ascontiguousarray(qb.reshape(8, 128).T)   # [128, 8]
    d["kb"] = np.ascontiguousarray(kb.reshape(8, 128).T)
    d["b1e"] = np.ascontiguousarray(b1e.reshape(NFF, 128).T)  # [128, 32]
    vb_pad = np.zeros(H * VW, f32)
    for h in range(H):
        vb_pad[h * VW + HS: h * VW + 2 * HS] = vb[h * HS:(h + 1) * HS]
    d["vbpad"] = vb_pad
    d["biasp"] = np.asarray(inputs["biasp"], f32)
    d["bias2"] = np.asarray(inputs["bias2"], f32)
    # causal 0/1 mask for diagonal S^T blocks (keep t_local >= s_local),
    # duplicated side by side so one op covers both heads of a pair
    mask = (np.arange(128)[None, :] >= np.arange(128)[:, None]).astype(npf8)
    d["mask8"] = np.ascontiguousarray(np.tile(mask, (1, 2)))   # [128, 256]
    flags = {
        "qb_nz": bool(np.any(qb != 0)),
        "kb_nz": bool(np.any(kb != 0)),
        "vb_nz": bool(np.any(vb != 0)),
        "b1_nz": bool(np.any(b1e != 0)),
        "biasp_nz": bool(np.any(d["biasp"] != 0)),
        "bias2_nz": bool(np.any(d["bias2"] != 0)),
    }
    return d, flags


# ----------------------------------------------------------------------------
# device kernel
# ----------------------------------------------------------------------------

def build(flags):
    import concourse.bass as bass
    import concourse.tile as tile
    from concourse import bacc, mybir

    f32 = mybir.dt.float32
    bf16 = mybir.dt.bfloat16
    f8 = mybir.dt.float8e4
    AF = mybir.ActivationFunctionType
    OP = mybir.AluOpType
    DR = mybir.MatmulPerfMode.DoubleRow

    nc = bacc.Bacc("TRN2", target_bir_lowering=False)

    xd = nc.dram_tensor("x", [T, C], f32, kind="ExternalInput")
    wq8d = nc.dram_tensor("wq8", [128, NCI, C], f8, kind="ExternalInput")
    wk8d = nc.dram_tensor("wk8", [128, NCI, C], f8, kind="ExternalInput")
    wv8d = nc.dram_tensor("wv8", [128, NCI, C], f8, kind="ExternalInput")
    wp8d = nc.dram_tensor("wp8", [128, NCI, C], f8, kind="ExternalInput")
    w18d = nc.dram_tensor("w18", [128, NCI, FF], f8, kind="ExternalInput")
    w28d = nc.dram_tensor("w28", [128, NFF, C], f8, kind="ExternalInput")
    qbd = nc.dram_tensor("qb", [128, 8], f32, kind="ExternalInput")
    kbd = nc.dram_tensor("kb", [128, 8], f32, kind="ExternalInput")
    b1ed = nc.dram_tensor("b1e", [128, NFF], f32, kind="ExternalInput")
    mask8d = nc.dram_tensor("mask8", [128, 256], f8, kind="ExternalInput")
    vbpd = nc.dram_tensor("vbpad", [H * VW], f32, kind="ExternalInput")
    biaspd = nc.dram_tensor("biasp", [C], f32, kind="ExternalInput")
    bias2d = nc.dram_tensor("bias2", [C], f32, kind="ExternalInput")
    outd = nc.dram_tensor("out", [T, C], f32, kind="ExternalOutput")

    def bcast_dram_row(vec_ap, n):
        return bass.AP(tensor=vec_ap.tensor, offset=vec_ap.offset,
                       ap=[[0, 128], [1, n]])

    def dma_chunked(dst, src, dim1):
        # one DMA per dim1 index so the loads spread across queues
        for i in range(dim1):
            nc.sync.dma_start(dst[:, i, :], src[:, i, :])

    with tile.TileContext(nc) as tc, \
         tc.tile_pool(name="consts", bufs=1) as consts, \
         tc.tile_pool(name="xpool", bufs=1) as xpool, \
         tc.tile_pool(name="hpool", bufs=1) as hpool, \
         tc.tile_pool(name="ln_tmp", bufs=3) as ln_tmp:

        # ---- constants ----
        from concourse.masks import make_identity
        ident8 = consts.tile([128, 128], f8, name="ident8")
        make_identity(nc, ident8[:])
        mask8_sb = consts.tile([128, 2, 128], f8, name="mask8_sb")
        nc.sync.dma_start(mask8_sb[:].rearrange("p e t -> p (e t)"),
                          mask8d[:, :])
        eps_sb = consts.tile([128, 1], f32, name="eps_sb")
        nc.vector.memset(eps_sb[:], EPS)
        if flags["qb_nz"]:
            qb_sb = consts.tile([128, 8], f32, name="qb_sb")
            nc.sync.dma_start(qb_sb[:], qbd[:, :])
        if flags["kb_nz"]:
            kb_sb = consts.tile([128, 8], f32, name="kb_sb")
            nc.sync.dma_start(kb_sb[:], kbd[:, :])
        if flags["b1_nz"]:
            b1e_sb = consts.tile([128, NFF], f32, name="b1e_sb")
            nc.sync.dma_start(b1e_sb[:], b1ed[:, :])
        if flags["vb_nz"]:
            vb_sb = consts.tile([128, H * VW], f32, name="vb_sb")
            nc.sync.dma_start(vb_sb[:], bcast_dram_row(vbpd[:], H * VW))
        if flags["biasp_nz"]:
            bp_sb = consts.tile([128, C], f32, name="bp_sb")
            nc.sync.dma_start(bp_sb[:], bcast_dram_row(biaspd[:], C))
        if flags["bias2_nz"]:
            b2_sb = consts.tile([128, C], f32, name="b2_sb")
            nc.sync.dma_start(b2_sb[:], bcast_dram_row(bias2d[:], C))

        # ---- x tiles (persist; become xnew, then the output) ----
        x_sb = []
        for t in range(NT):
            xt = xpool.tile([128, C], f32, name=f"x{t}")
            nc.sync.dma_start(xt[:, 0:512], xd[128 * t:128 * (t + 1), 0:512])
            nc.sync.dma_start(xt[:, 512:1024],
                              xd[128 * t:128 * (t + 1), 512:1024])
            x_sb.append(xt)

        # hT in two t-halves so consumers can start after 4 t-tiles
        hT = [hpool.tile([128, NCI, 512], f8, name=f"hT{i}") for i in range(2)]

        def ln_tile(t):
            """LN stats + normalize for x tile t -> fp8 h tile (returns it)."""
            xt = x_sb[t]
            stats = ln_tmp.tile([128, 2, 6], f32, tag="lnstats")
            nc.vector.bn_stats(stats[:, 0, :], xt[:, 0:512])
            nc.vector.bn_stats(stats[:, 1, :], xt[:, 512:1024])
            mv = ln_tmp.tile([128, 2], f32, tag="lnmv")
            nc.vector.bn_aggr(mv[:], stats[:])
            rstd = ln_tmp.tile([128, 1], f32, tag="lnrstd")
            nc.scalar.activation(rstd[:], mv[:, 1:2], AF.Sqrt, bias=eps_sb[:])
            nc.vector.reciprocal(rstd[:], rstd[:])
            nmr = ln_tmp.tile([128, 1], f32, tag="lnnmr")
            nc.vector.tensor_scalar(nmr[:], mv[:, 0:1], rstd[:], -1.0,
                                    OP.mult, OP.mult)
            ht = ln_tmp.tile([128, C], f8, tag="lnh")
            nc.scalar.activation(ht[:], xt[:], AF.Identity,
                                 bias=nmr[:], scale=rstd[:])
            return ht

        def ln_transpose(t, ht, ps_tr):
            """PE-transpose h tile t into hT[t // 4] (batched evacuation)."""
            ptr = ps_tr.tile([128, NCI, 128, 2], f8, tag="tr")
            for c in range(NCI):
                nc.tensor.transpose(ptr[:, c, :, 0],
                                    ht[:, 128 * c:128 * (c + 1)], ident8[:])
            half, tl = divmod(t, 4)
            nc.vector.tensor_copy(hT[half][:, :, 128 * tl:128 * (tl + 1)],
                                  ptr[:, :, :, 0])

        with tc.tile_pool(name="wpp", bufs=1) as wpp, \
             tc.tile_pool(name="w1p", bufs=1) as w1p, \
             tc.tile_pool(name="att", bufs=1) as att, \
             tc.tile_pool(name="qkv", bufs=1) as qkv:
            attT8 = att.tile([128, NCI, T], f8, name="attT8")
            qT_sb = [qkv.tile([128, T], f8, name=f"qT{p}") for p in range(8)]
            kT_sb = [qkv.tile([128, T], f8, name=f"kT{p}") for p in range(8)]
            v8 = qkv.tile([128, NT, H, VW], f8, name="v8")
            # ones in cols 0..63 -> avp rows 0..63 all hold the softmax sums
            nc.gpsimd.memset(v8[:, :, :, 0:HS], 1.0)

            with tc.tile_pool(name="wqkv", bufs=1) as wqkv:
                wq8 = wqkv.tile([128, NCI, C], f8, name="wq8")
                dma_chunked(wq8, wq8d, NCI)
                wk8 = wqkv.tile([128, NCI, C], f8, name="wk8")
                dma_chunked(wk8, wk8d, NCI)
                wv8 = wqkv.tile([128, NCI, C], f8, name="wv8")
                dma_chunked(wv8, wv8d, NCI)

                # ====================== phase 1: LN1 ========================
                with tc.tile_pool(name="ps_tr1", bufs=2, space="PSUM") as ps_tr:
                    for t in range(NT):
                        ln_transpose(t, ln_tile(t), ps_tr)

                # ====================== phase 2: QKV ========================
                with tc.tile_pool(name="ps_qkv", bufs=2, space="PSUM") as ps_qkv:
                    for (w8, dst, bias_nm, b_nz) in (
                            (wq8, qT_sb, "qb", flags["qb_nz"]),
                            (wk8, kT_sb, "kb", flags["kb_nz"])):
                        for p in range(8):
                            ps = ps_qkv.tile([128, T], f32, tag="mm")
                            for half in (0, 1):
                                for cp in range(4):
                                    nc.tensor.matmul(
                                        ps[:, 512 * half:512 * half + 512],
                                        lhsT=w8[:, 2 * cp:2 * cp + 2,
                                                128 * p:128 * (p + 1)],
                                        rhs=hT[half][:, 2 * cp:2 * cp + 2, :],
                                        start=(cp == 0), stop=(cp == 3),
                                        perf_mode=DR)
                            if b_nz:
                                bias_sb = qb_sb if bias_nm == "qb" else kb_sb
                                nc.scalar.activation(dst[p][:], ps[:],
                                                     AF.Identity,
                                                     bias=bias_sb[:, p:p + 1])
                            else:
                                nc.scalar.activation(dst[p][:], ps[:],
                                                     AF.Identity)
                    # v: [t-part, head-major d]
                    for t in range(NT):
                        half, tl = divmod(t, 4)
                        ps = ps_qkv.tile([128, T], f32, tag="mm")
                        for off in (0, 512):
                            for cp in range(4):
                                nc.tensor.matmul(
                                    ps[:, off:off + 512],
                                    lhsT=hT[half][:, 2 * cp:2 * cp + 2,
                                                  128 * tl:128 * (tl + 1)],
                                    rhs=wv8[:, 2 * cp:2 * cp + 2,
                                            off:off + 512],
                                    start=(cp == 0), stop=(cp == 3),
                                    perf_mode=DR)
                        ps3 = ps[:].rearrange("p (h d) -> p h d", d=HS)
                        if flags["vb_nz"]:
                            vb3 = vb_sb[:].rearrange("p (h w) -> p h w", w=VW)
                            nc.vector.tensor_tensor(v8[:, t, :, HS:2 * HS],
                                                    ps3,
                                                    vb3[:, :, HS:2 * HS],
                                                    OP.add)
                        else:
                            nc.vector.tensor_copy(v8[:, t, :, HS:2 * HS], ps3)
            # wqkv pool closed - wq/wk/wv freed before attention

            # ===================== phase 3: attention =======================
            # prefetch Wp and W1 while attention runs
            wp8 = wpp.tile([128, NCI, C], f8, name="wp8")
            dma_chunked(wp8, wp8d, NCI)
            w18 = w1p.tile([128, NCI, FF], f8, name="w18")
            dma_chunked(w18, w18d, NCI)

            with tc.tile_pool(name="exp_pool", bufs=2) as exp_pool, \
                 tc.tile_pool(name="r_pool", bufs=2) as r_pool, \
                 tc.tile_pool(name="rr_pool", bufs=2) as rr_pool, \
                 tc.tile_pool(name="ps_st", bufs=2, space="PSUM") as ps_st, \
                 tc.tile_pool(name="ps_av", bufs=2, space="PSUM") as ps_av:
                for p8 in range(8):
                    ex = exp_pool.tile([128, NT, 2, T], f8, tag="exp",
                                       name=f"exp{p8}")
                    # zero-fill the non-causal 128-col blocks of odd s-tiles
                    # (DoubleRow pairs s-tiles (2a, 2a+1); the shared window
                    # starts at t=256a)
                    for a in range(4):
                        for e in (0, 1):
                            nc.gpsimd.memset(
                                ex[:, 2 * a + 1, e, 256 * a:256 * a + 128],
                                0.0)
                    avp = {e: ps_av.tile([VW, T], f32, tag="av",
                                         name=f"av{2 * p8 + e}")
                           for e in (0, 1)}
                    for a in range(4):
                        for j in (2 * a, 2 * a + 1):
                            W = T - 128 * j
                            for e in (0, 1):
                                po = 64 * e
                                st = ps_st.tile([128, T], f32, tag="st",
                                                name=f"st{p8}_{j}_{e}")
                                for off in range(0, W, 512):
                                    w = min(512, W - off)
                                    nc.tensor.matmul(
                                        st[:, off:off + w],
                                        lhsT=kT_sb[p8][po:po + 64,
                                                       128 * j:128 * (j + 1)],
                                        rhs=qT_sb[p8][po:po + 64,
                                                      128 * j + off:
                                                      128 * j + off + w],
                                        start=True, stop=True)
                                nc.scalar.activation(
                                    ex[:, j, e, 128 * j:T], st[:, 0:W],
                                    AF.Exp, scale=SM_SCALE)
                            # causal mask on both heads' diagonal blocks
                            nc.gpsimd.tensor_tensor(
                                ex[:, j, 0:2, 128 * j:128 * (j + 1)],
                                ex[:, j, 0:2, 128 * j:128 * (j + 1)],
                                mask8_sb[:], OP.mult)
                        # AV contribution of s-tile pair a (both heads)
                        for e in (0, 1):
                            h = 2 * p8 + e
                            for off in (0, 512):
                                if 256 * a >= off + 512:
                                    continue
                                aa = [q for q in range(4) if 256 * q < off + 512]
                                lo = max(off, 256 * a)
                                nc.tensor.matmul(
                                    avp[e][0:VW, lo:off + 512],
                                    lhsT=v8[:, 2 * a:2 * a + 2, h, 0:VW],
                                    rhs=ex[:, 2 * a:2 * a + 2, e,
                                           lo:off + 512],
                                    start=(a == aa[0]), stop=(a == aa[-1]),
                                    perf_mode=DR)
                    for e in (0, 1):
                        # r = 1/sums (avp row 0); attT = att_unnorm * r (fp8).
                        # reciprocal_approx_fast mis-executes on 1-partition
                        # APs, so recip the whole avp tile and use row 0.
                        r_sb = r_pool.tile([VW, T], f32, tag="r",
                                           name=f"r{2 * p8 + e}")
                        nc.vector.reciprocal_approx_fast(r_sb[:],
                                                         avp[e][0:VW, :])
                        rr = rr_pool.tile([128, T], f32, tag="rr",
                                          name=f"rr{2 * p8 + e}")
                        nc.gpsimd.partition_broadcast(rr[:], r_sb[0:1, :],
                                                      channels=128)
                        nc.vector.tensor_tensor(
                            attT8[64 * e:64 * e + 64, p8, :],
                            avp[e][HS:2 * HS, :], rr[0:64, :], OP.mult)
        # qkv + attention pools closed here

            # ============== phase 4+5: proj + LN2 (staggered) ===============
            with tc.tile_pool(name="w2p", bufs=1) as w2p, \
                 tc.tile_pool(name="f1pool", bufs=1) as f1pool:
                w28 = w2p.tile([128, NFF, C], f8, name="w28")
                for i in range(8):
                    nc.sync.dma_start(w28[:, 4 * i:4 * (i + 1), :],
                                      w28d[:, 4 * i:4 * (i + 1), :])

                with tc.tile_pool(name="ps_proj", bufs=2, space="PSUM") as ps_proj, \
                     tc.tile_pool(name="ps_tr2", bufs=2, space="PSUM") as ps_tr:
                    pending = None  # (t, ht) awaiting PE transposes
                    for t in range(NT):
                        ps = ps_proj.tile([128, C], f32, tag="mm")
                        for off in (0, 512):
                            for cp in range(4):
                                nc.tensor.matmul(
                                    ps[:, off:off + 512],
                                    lhsT=attT8[:, 2 * cp:2 * cp + 2,
                                               128 * t:128 * (t + 1)],
                                    rhs=wp8[:, 2 * cp:2 * cp + 2,
                                            off:off + 512],
                                    start=(cp == 0), stop=(cp == 3),
                                    perf_mode=DR)
                        if pending is not None:
                            ln_transpose(pending[0], pending[1], ps_tr)
                        nc.vector.tensor_tensor(x_sb[t][:], ps[:], x_sb[t][:],
                                                OP.add)
                        if flags["biasp_nz"]:
                            nc.vector.tensor_tensor(x_sb[t][:], x_sb[t][:],
                                                    bp_sb[:], OP.add)
                        pending = (t, ln_tile(t))
                    ln_transpose(pending[0], pending[1], ps_tr)

                # ======================= phase 6: FFN =======================
                f1 = [f1pool.tile([128, NFF, 512], f8, name=f"f1_{i}")
                      for i in range(2)]
                with tc.tile_pool(name="ps_f1", bufs=2, space="PSUM") as ps_f1, \
                     tc.tile_pool(name="ps_y2", bufs=1, space="PSUM") as ps_y2:

                    def ffn1_half(half):
                        for f in range(NFF):
                            ps = ps_f1.tile([128, 512], f32, tag="mm")
                            for cp in range(4):
                                nc.tensor.matmul(
                                    ps[:],
                                    lhsT=w18[:, 2 * cp:2 * cp + 2,
                                             128 * f:128 * (f + 1)],
                                    rhs=hT[half][:, 2 * cp:2 * cp + 2, :],
                                    start=(cp == 0), stop=(cp == 3),
                                    perf_mode=DR)
                            if flags["b1_nz"]:
                                nc.scalar.activation(f1[half][:, f, :], ps[:],
                                                     AF.Relu,
                                                     bias=b1e_sb[:, f:f + 1])
                            elif f % 2 == 0:
                                nc.scalar.activation(f1[half][:, f, :], ps[:],
                                                     AF.Relu)
                            else:
                                nc.vector.tensor_scalar_max(f1[half][:, f, :],
                                                            ps[:], 0.0)

                    def ffn2_group(tg):
                        for off in (0, 512):
                            trange = range(4 * tg, 4 * tg + 4)
                            y2 = {t: ps_y2.tile([128, 512], f32,
                                                tag=f"y2_{t % 4}",
                                                name=f"y2_{off}_{t}")
                                  for t in trange}
                            for fp in range(NFF // 2):
                                for t in trange:
                                    tl = t % 4
                                    nc.tensor.matmul(
                                        y2[t][:],
                                        lhsT=f1[tg][:, 2 * fp:2 * fp + 2,
                                                    128 * tl:128 * (tl + 1)],
                                        rhs=w28[:, 2 * fp:2 * fp + 2,
                                                off:off + 512],
                                        start=(fp == 0), stop=(fp == 15),
                                        perf_mode=DR)
                            for t in trange:
                                nc.vector.tensor_tensor(
                                    x_sb[t][:, off:off + 512], y2[t][:],
                                    x_sb[t][:, off:off + 512], OP.add)
                                if flags["bias2_nz"]:
                                    nc.vector.tensor_tensor(
                                        x_sb[t][:, off:off + 512],
                                        x_sb[t][:, off:off + 512],
                                        b2_sb[:, off:off + 512], OP.add)
                                if off == 512:
                                    nc.sync.dma_start(
                                        outd[128 * t:128 * (t + 1), 0:512],
                                        x_sb[t][:, 0:512])
                                    nc.sync.dma_start(
                                        outd[128 * t:128 * (t + 1), 512:1024],
                                        x_sb[t][:, 512:1024])

                    ffn1_half(0)   # f1[0] = all f for t 0..511
                    ffn2_group(0)  # FFN2 on t 0..511 chases FFN1 half 1
                    ffn1_half(1)
                    ffn2_group(1)
    nc.compile()
    return nc


def _get_nc(flags):
    key = tuple(sorted(flags.items()))
    if key not in _CACHE:
        _CACHE[key] = build(flags)
    return _CACHE[key]


# ----------------------------------------------------------------------------
# public entry point
# ----------------------------------------------------------------------------

def kernel(**inputs):
    from concourse import bass_utils
    x = np.asarray(inputs["x"], np.float32)
    d, flags = _prep(inputs)
    nc = _get_nc(flags)
    in_maps = []
    for b in range(B):
        m = dict(d)
        m["x"] = np.ascontiguousarray(x[b])
        in_maps.append(m)
    res = bass_utils.run_bass_kernel_spmd(nc, in_maps, core_ids=list(range(B)))
    out = np.stack([r["out"] for r in res.results]).astype(np.float32)
    return out


# revision 32
# speedup vs baseline: 1.2893x; 1.2893x over previous
"""Trainium2 Bass kernel for a dense transformer block (B=8, T=1024, C=1024, H=16, FF=4096).

Sharding: data-parallel over batch - one batch element per NeuronCore (8 cores),
no collectives.

Speed levers vs the bf16 baseline (631 us):
  * fp8e4m3 DoubleRow matmuls for QKV, proj, FFN1, FFN2 and the attention AV
    contraction: each PE instruction contracts 256 rows instead of 128 at the
    same column rate, halving PE time for the big GEMMs. The fake-quantized
    weights are integers k*2^-8 with |k|<=8 - EXACT in fp8e4m3; only
    activations pick up ~2% rounding noise (end-to-end rel err ~9.6e-3,
    gate 2e-2).
  * Engine balance: during attention ACT runs only Exp (no act-table swaps);
    DVE does LN stats, transpose evacuation, softmax normalize, residual
    adds; GpSimd does causal masks, zero-fills and r broadcasts.
  * Pipelining: weight DMAs split into chunks across queues; QKV starts on
    the first t-half of LN1; S/exp and AV interleave per s-tile pair; LN2
    transposes stagger between proj matmuls; FFN1 runs half-major so FFN2
    chases it; outputs stream out per t-group.

Layouts (per core):
  hT    [128, 8, 512] fp8 x2 : hT[p, i, t] = h[t, 128*i + p] (t-halves; LN1 then LN2)
  w*8   [128, K/128, M] fp8  : w[p, i, m] = W[m, 128*i + p] (DoubleRow pairs = dim1)
  qT/kT [128, 1024] fp8 per head-pair (rows 64e+d)
  v8    [128, 8, 16, 128] fp8: cols 0-63 ones (sums -> avp rows 0-63), 64-127 = v
  exp8  [128, 8, 2, 1024] fp8 per pair: exp8[s, j, e, t] = exp(S[t, 128j+s]) unnorm
  attT8 [128, 8, 1024] fp8   : attT8[64e+d, p8, t] = normalized att
  f1    [128, 32, 512] fp8 x2: f1[p, f, t] = relu(ffn1)[t, 128f+p] (t-halves)
"""

import os
import numpy as np
import ml_dtypes

DEBUG = bool(int(os.environ.get("BASSDBG", "0")))

B, T, C, H = 8, 1024, 1024, 16
HS = C // H          # 64
FF = 4 * C           # 4096
EPS = 1e-5
NT = T // 128        # 8 t-tiles
NCI = C // 128       # 8 c-tiles
NFF = FF // 128      # 32 ff-tiles
VW = 128             # per-head stride in v8: cols 0-63 ones (sums on avp row 0
                     # for partition_broadcast), v in cols 64-127 (partition
                     # ranges of 64 must start at 0 or 64)
SM_SCALE = 1.0 / 32.0  # C ** -0.5

_CACHE = {}

npf8 = ml_dtypes.float8_e4m3  # TRN fp8e4 (max 240)


# ----------------------------------------------------------------------------
# host-side math (exact reference semantics)
# ----------------------------------------------------------------------------

def _quant_weight(W, e, b):
    W = np.asarray(W, np.float32)
    e = np.asarray(e, np.float32)
    b = np.asarray(b, np.float32)
    b_rel = np.maximum(b, 0.0)
    mn = np.where(b_rel > 0, -(2.0 ** (b_rel - 1)), 0.0)
    mx = np.where(b_rel > 0, 2.0 ** (b_rel - 1) - 1.0, 0.0)
    qw = np.clip((2.0 ** (-e)) * W, mn, mx)
    w = np.round(qw)  # round-half-even, same as jnp.round
    return ((2.0 ** e) * w).astype(np.float32)


def _dr_layout(WT):
    """[K, M] -> [128, K//128, M] fp8 DoubleRow layout: out[p, i, m] = WT[128i+p, m]."""
    K, M = WT.shape
    return np.ascontiguousarray(
        WT.reshape(K // 128, 128, M).transpose(1, 0, 2)).astype(npf8)


def _prep(inputs):
    f32 = np.float32
    g1 = np.asarray(inputs["g1"], f32)
    be1 = np.asarray(inputs["be1"], f32)
    g2 = np.asarray(inputs["g2"], f32)
    be2 = np.asarray(inputs["be2"], f32)

    Wq = _quant_weight(inputs["Wq"], inputs["eq"], inputs["bq"])  # [H,HS,C]
    Wk = _quant_weight(inputs["Wk"], inputs["ek"], inputs["bk"])
    Wv = _quant_weight(inputs["Wv"], inputs["ev"], inputs["bv"])
    Wp = _quant_weight(inputs["Wp"], inputs["ep"], inputs["bp"])  # [C,C]
    W1 = _quant_weight(inputs["W1"], inputs["e1"], inputs["b1"])  # [FF,C]
    W2 = _quant_weight(inputs["W2"], inputs["e2"], inputs["b2"])  # [C,FF]

    # fold LN gains into the adjacent weights (identity when g == 1, so the
    # fp8 cast of the quantized weights stays exact in that case)
    Wqf = (Wq * g1[None, None, :]).reshape(H * HS, C)
    Wkf = (Wk * g1[None, None, :]).reshape(H * HS, C)
    Wvf = (Wv * g1[None, None, :]).reshape(H * HS, C)
    W1f = W1 * g2[None, :]

    d = {
        "wq8": _dr_layout(Wqf.T), "wk8": _dr_layout(Wkf.T), "wv8": _dr_layout(Wvf.T),
        "wp8": _dr_layout(np.ascontiguousarray(Wp.T)),
        "w18": _dr_layout(np.ascontiguousarray(W1f.T)),
        "w28": _dr_layout(np.ascontiguousarray(W2.T)),
    }
    # biases from LN betas routed through the projections
    qb = (Wqf @ be1).astype(f32)   # [H*HS]
    kb = (Wkf @ be1).astype(f32)
    vb = (Wvf @ be1).astype(f32)
    b1e = (np.asarray(inputs["bias1"], f32) + W1 @ be2).astype(f32)  # [FF]
    d["qb"] = np.ascontiguousarray(qb.reshape(8, 128).T)   # [128, 8]
    d["kb"] = np.ascontiguousarray(kb.reshape(8, 128).T)
    d["b1e"] = np.ascontiguousarray(b1e.reshape(NFF, 128).T)  # [128, 32]
    vb_pad = np.zeros(H * VW, f32)
    for h in range(H):
        vb_pad[h * VW + HS: h * VW + 2 * HS] = vb[h * HS:(h + 1) * HS]
    d["vbpad"] = vb_pad
    d["biasp"] = np.asarray(inputs["biasp"], f32)
    d["bias2"] = np.asarray(inputs["bias2"], f32)
    # causal 0/1 mask for diagonal S^T blocks (keep t_local >= s_local),
    # duplicated side by side so one op covers both heads of a pair
    mask = (np.arange(128)[None, :] >= np.arange(128)[:, None]).astype(npf8)
    d["mask8"] = np.ascontiguousarray(np.tile(mask, (1, 2)))   # [128, 256]
    flags = {
        "qb_nz": bool(np.any(qb != 0)),
        "kb_nz": bool(np.any(kb != 0)),
        "vb_nz": bool(np.any(vb != 0)),
        "b1_nz": bool(np.any(b1e != 0)),
        "biasp_nz": bool(np.any(d["biasp"] != 0)),
        "bias2_nz": bool(np.any(d["bias2"] != 0)),
    }
    return d, flags


# ----------------------------------------------------------------------------
# device kernel
# ----------------------------------------------------------------------------

def build(flags):
    import concourse.bass as bass
    import concourse.tile as tile
    from concourse import bacc, mybir

    f32 = mybir.dt.float32
    bf16 = mybir.dt.bfloat16
    f8 = mybir.dt.float8e4
    AF = mybir.ActivationFunctionType
    OP = mybir.AluOpType
    DR = mybir.MatmulPerfMode.DoubleRow

    nc = bacc.Bacc("TRN2", target_bir_lowering=False)

    xd = nc.dram_tensor("x", [T, C], f32, kind="ExternalInput")
    wq8d = nc.dram_tensor("wq8", [128, NCI, C], f8, kind="ExternalInput")
    wk8d = nc.dram_tensor("wk8", [128, NCI, C], f8, kind="ExternalInput")
    wv8d = nc.dram_tensor("wv8", [128, NCI, C], f8, kind="ExternalInput")
    wp8d = nc.dram_tensor("wp8", [128, NCI, C], f8, kind="ExternalInput")
    w18d = nc.dram_tensor("w18", [128, NCI, FF], f8, kind="ExternalInput")
    w28d = nc.dram_tensor("w28", [128, NFF, C], f8, kind="ExternalInput")
    qbd = nc.dram_tensor("qb", [128, 8], f32, kind="ExternalInput")
    kbd = nc.dram_tensor("kb", [128, 8], f32, kind="ExternalInput")
    b1ed = nc.dram_tensor("b1e", [128, NFF], f32, kind="ExternalInput")
    mask8d = nc.dram_tensor("mask8", [128, 256], f8, kind="ExternalInput")
    vbpd = nc.dram_tensor("vbpad", [H * VW], f32, kind="ExternalInput")
    biaspd = nc.dram_tensor("biasp", [C], f32, kind="ExternalInput")
    bias2d = nc.dram_tensor("bias2", [C], f32, kind="ExternalInput")
    outd = nc.dram_tensor("out", [T, C], f32, kind="ExternalOutput")

    def bcast_dram_row(vec_ap, n):
        return bass.AP(tensor=vec_ap.tensor, offset=vec_ap.offset,
                       ap=[[0, 128], [1, n]])

    def dma_chunked(dst, src, dim1):
        # one DMA per dim1 index so the loads spread across queues
        for i in range(dim1):
            nc.sync.dma_start(dst[:, i, :], src[:, i, :])

    with tile.TileContext(nc) as tc, \
         tc.tile_pool(name="consts", bufs=1) as consts, \
         tc.tile_pool(name="xpool", bufs=1) as xpool, \
         tc.tile_pool(name="hpool", bufs=1) as hpool, \
         tc.tile_pool(name="ln_tmp", bufs=3) as ln_tmp:

        # ---- constants ----
        from concourse.masks import make_identity
        ident8 = consts.tile([128, 128], f8, name="ident8")
        make_identity(nc, ident8[:])
        mask8_sb = consts.tile([128, 2, 128], f8, name="mask8_sb")
        nc.sync.dma_start(mask8_sb[:].rearrange("p e t -> p (e t)"),
                          mask8d[:, :])
        eps_sb = consts.tile([128, 1], f32, name="eps_sb")
        nc.vector.memset(eps_sb[:], EPS)
        if flags["qb_nz"]:
            qb_sb = consts.tile([128, 8], f32, name="qb_sb")
            nc.sync.dma_start(qb_sb[:], qbd[:, :])
        if flags["kb_nz"]:
            kb_sb = consts.tile([128, 8], f32, name="kb_sb")
            nc.sync.dma_start(kb_sb[:], kbd[:, :])
        if flags["b1_nz"]:
            b1e_sb = consts.tile([128, NFF], f32, name="b1e_sb")
            nc.sync.dma_start(b1e_sb[:], b1ed[:, :])
        if flags["vb_nz"]:
            vb_sb = consts.tile([128, H * VW], f32, name="vb_sb")
            nc.sync.dma_start(vb_sb[:], bcast_dram_row(vbpd[:], H * VW))
        if flags["biasp_nz"]:
            bp_sb = consts.tile([128, C], f32, name="bp_sb")
            nc.sync.dma_start(bp_sb[:], bcast_dram_row(biaspd[:], C))
        if flags["bias2_nz"]:
            b2_sb = consts.tile([128, C], f32, name="b2_sb")
            nc.sync.dma_start(b2_sb[:], bcast_dram_row(bias2d[:], C))

        # ---- x tiles (persist; become xnew, then the output) ----
        x_sb = []
        for t in range(NT):
            xt = xpool.tile([128, C], f32, name=f"x{t}")
            nc.sync.dma_start(xt[:, 0:512], xd[128 * t:128 * (t + 1), 0:512])
            nc.sync.dma_start(xt[:, 512:1024],
                              xd[128 * t:128 * (t + 1), 512:1024])
            x_sb.append(xt)

        # hT in two t-halves so consumers can start after 4 t-tiles
        hT = [hpool.tile([128, NCI, 512], f8, name=f"hT{i}") for i in range(2)]

        def ln_tile(t):
            """LN stats + normalize for x tile t -> fp8 h tile (returns it)."""
            xt = x_sb[t]
            stats = ln_tmp.tile([128, 2, 6], f32, tag="lnstats")
            nc.vector.bn_stats(stats[:, 0, :], xt[:, 0:512])
            nc.vector.bn_stats(stats[:, 1, :], xt[:, 512:1024])
            mv = ln_tmp.tile([128, 2], f32, tag="lnmv")
            nc.vector.bn_aggr(mv[:], stats[:])
            rstd = ln_tmp.tile([128, 1], f32, tag="lnrstd")
            nc.scalar.activation(rstd[:], mv[:, 1:2], AF.Sqrt, bias=eps_sb[:])
            nc.vector.reciprocal(rstd[:], rstd[:])
            nmr = ln_tmp.tile([128, 1], f32, tag="lnnmr")
            nc.vector.tensor_scalar(nmr[:], mv[:, 0:1], rstd[:], -1.0,
                                    OP.mult, OP.mult)
            ht = ln_tmp.tile([128, C], f8, tag="lnh")
            nc.scalar.activation(ht[:], xt[:], AF.Identity,
                                 bias=nmr[:], scale=rstd[:])
            return ht

        def ln_transpose(t, ht, ps_tr):
            """PE-transpose h tile t into hT[t // 4] (batched evacuation)."""
            ptr = ps_tr.tile([128, NCI, 128, 2], f8, tag="tr")
            for c in range(NCI):
                nc.tensor.transpose(ptr[:, c, :, 0],
                                    ht[:, 128 * c:128 * (c + 1)], ident8[:])
            half, tl = divmod(t, 4)
            nc.vector.tensor_copy(hT[half][:, :, 128 * tl:128 * (tl + 1)],
                                  ptr[:, :, :, 0])

        with tc.tile_pool(name="wpp", bufs=1) as wpp, \
             tc.tile_pool(name="w1p", bufs=1) as w1p, \
             tc.tile_pool(name="att", bufs=1) as att, \
             tc.tile_pool(name="qkv", bufs=1) as qkv:
            attT8 = att.tile([128, NCI, T], f8, name="attT8")
            qT_sb = [qkv.tile([128, T], f8, name=f"qT{p}") for p in range(8)]
            kT_sb = [qkv.tile([128, T], f8, name=f"kT{p}") for p in range(8)]
            v8 = qkv.tile([128, NT, H, VW], f8, name="v8")
            # ones in cols 0..63 -> avp rows 0..63 all hold the softmax sums
            nc.gpsimd.memset(v8[:, :, :, 0:HS], 1.0)

            with tc.tile_pool(name="wqkv", bufs=1) as wqkv:
                wq8 = wqkv.tile([128, NCI, C], f8, name="wq8")
                dma_chunked(wq8, wq8d, NCI)
                wk8 = wqkv.tile([128, NCI, C], f8, name="wk8")
                dma_chunked(wk8, wk8d, NCI)
                wv8 = wqkv.tile([128, NCI, C], f8, name="wv8")
                dma_chunked(wv8, wv8d, NCI)

                # ====================== phase 1: LN1 ========================
                with tc.tile_pool(name="ps_tr1", bufs=2, space="PSUM") as ps_tr:
                    for t in range(NT):
                        ln_transpose(t, ln_tile(t), ps_tr)

                # ====================== phase 2: QKV ========================
                with tc.tile_pool(name="ps_qkv", bufs=2, space="PSUM") as ps_qkv:
                    for (w8, dst, bias_nm, b_nz) in (
                            (wq8, qT_sb, "qb", flags["qb_nz"]),
                            (wk8, kT_sb, "kb", flags["kb_nz"])):
                        for p in range(8):
                            ps = ps_qkv.tile([128, T], f32, tag="mm")
                            for half in (0, 1):
                                for cp in range(4):
                                    nc.tensor.matmul(
                                        ps[:, 512 * half:512 * half + 512],
                                        lhsT=w8[:, 2 * cp:2 * cp + 2,
                                                128 * p:128 * (p + 1)],
                                        rhs=hT[half][:, 2 * cp:2 * cp + 2, :],
                                        start=(cp == 0), stop=(cp == 3),
                                        perf_mode=DR)
                            if b_nz:
                                bias_sb = qb_sb if bias_nm == "qb" else kb_sb
                                nc.scalar.activation(dst[p][:], ps[:],
                                                     AF.Identity,
                                                     bias=bias_sb[:, p:p + 1])
                            else:
                                nc.scalar.activation(dst[p][:], ps[:],
                                                     AF.Identity)
                    # v: [t-part, head-major d]
                    for t in range(NT):
                        half, tl = divmod(t, 4)
                        ps = ps_qkv.tile([128, T], f32, tag="mm")
                        for off in (0, 512):
                            for cp in range(4):
                                nc.tensor.matmul(
                                    ps[:, off:off + 512],
                                    lhsT=hT[half][:, 2 * cp:2 * cp + 2,
                                                  128 * tl:128 * (tl + 1)],
                                    rhs=wv8[:, 2 * cp:2 * cp + 2,
                                            off:off + 512],
                                    start=(cp == 0), stop=(cp == 3),
                                    perf_mode=DR)
                        ps3 = ps[:].rearrange("p (h d) -> p h d", d=HS)
                        if flags["vb_nz"]:
                            vb3 = vb_sb[:].rearrange("p (h w) -> p h w", w=VW)
                            nc.vector.tensor_tensor(v8[:, t, :, HS:2 * HS],
                                                    ps3,
                                                    vb3[:, :, HS:2 * HS],
                                                    OP.add)
                        else:
                            nc.vector.tensor_copy(v8[:, t, :, HS:2 * HS], ps3)
            # wqkv pool closed - wq/wk/wv freed before attention

            # ===================== phase 3: attention =======================
            # prefetch Wp and W1 while attention runs
            wp8 = wpp.tile([128, NCI, C], f8, name="wp8")
            dma_chunked(wp8, wp8d, NCI)
            w18 = w1p.tile([128, NCI, FF], f8, name="w18")
            dma_chunked(w18, w18d, NCI)

            with tc.tile_pool(name="exp_pool", bufs=2) as exp_pool, \
                 tc.tile_pool(name="r_pool", bufs=2) as r_pool, \
                 tc.tile_pool(name="rr_pool", bufs=2) as rr_pool, \
                 tc.tile_pool(name="ps_st", bufs=2, space="PSUM") as ps_st, \
                 tc.tile_pool(name="ps_av", bufs=2, space="PSUM") as ps_av:
                for p8 in range(8):
                    ex = exp_pool.tile([128, NT, 2, T], f8, tag="exp",
                                       name=f"exp{p8}")
                    # zero-fill the non-causal 128-col blocks of odd s-tiles
                    # (DoubleRow pairs s-tiles (2a, 2a+1); the shared window
                    # starts at t=256a)
                    for a in range(4):
                        for e in (0, 1):
                            nc.gpsimd.memset(
                                ex[:, 2 * a + 1, e, 256 * a:256 * a + 128],
                                0.0)
                    for j in range(NT):
                        W = T - 128 * j
                        for e in (0, 1):
                            po = 64 * e
                            st = ps_st.tile([128, T], f32, tag="st",
                                            name=f"st{p8}_{j}_{e}")
                            for off in range(0, W, 512):
                                w = min(512, W - off)
                                nc.tensor.matmul(
                                    st[:, off:off + w],
                                    lhsT=kT_sb[p8][po:po + 64,
                                                   128 * j:128 * (j + 1)],
                                    rhs=qT_sb[p8][po:po + 64,
                                                  128 * j + off:
                                                  128 * j + off + w],
                                    start=True, stop=True)
                            nc.scalar.activation(
                                ex[:, j, e, 128 * j:T], st[:, 0:W],
                                AF.Exp, scale=SM_SCALE)
                        # causal mask on both heads' diagonal blocks
                        nc.vector.tensor_tensor(
                            ex[:, j, 0:2, 128 * j:128 * (j + 1)],
                            ex[:, j, 0:2, 128 * j:128 * (j + 1)],
                            mask8_sb[:], OP.mult)
                    for e in (0, 1):
                        h = 2 * p8 + e
                        avp_e = ps_av.tile([VW, T], f32, tag="av",
                                           name=f"av{h}")
                        for off in (0, 512):
                            aa = [q for q in range(4) if 256 * q < off + 512]
                            for a in aa:
                                lo = max(off, 256 * a)
                                nc.tensor.matmul(
                                    avp_e[0:VW, lo:off + 512],
                                    lhsT=v8[:, 2 * a:2 * a + 2, h, 0:VW],
                                    rhs=ex[:, 2 * a:2 * a + 2, e,
                                           lo:off + 512],
                                    start=(a == aa[0]), stop=(a == aa[-1]),
                                    perf_mode=DR)
                        # r = 1/sums (avp row 0); attT = att_unnorm * r
                        # (fp8). reciprocal_approx_fast mis-executes on
                        # 1-partition APs: recip the whole avp, use row 0.
                        if True:
                            r_sb = r_pool.tile([VW, T], f32, tag="r",
                                               name=f"r{2 * p8 + e}")
                            nc.vector.reciprocal_approx_fast(r_sb[:],
                                                             avp_e[0:VW, :])
                            rr = rr_pool.tile([128, T], f32, tag="rr",
                                              name=f"rr{2 * p8 + e}")
                            nc.gpsimd.partition_broadcast(rr[:], r_sb[0:1, :],
                                                          channels=128)
                            nc.vector.tensor_tensor(
                                attT8[64 * e:64 * e + 64, p8, :],
                                avp_e[HS:2 * HS, :], rr[0:64, :], OP.mult)
        # qkv + attention pools closed here

            # ============== phase 4+5: proj + LN2 (staggered) ===============
            with tc.tile_pool(name="w2p", bufs=1) as w2p, \
                 tc.tile_pool(name="f1pool", bufs=1) as f1pool:
                w28 = w2p.tile([128, NFF, C], f8, name="w28")
                for i in range(8):
                    nc.sync.dma_start(w28[:, 4 * i:4 * (i + 1), :],
                                      w28d[:, 4 * i:4 * (i + 1), :])

                with tc.tile_pool(name="ps_proj", bufs=2, space="PSUM") as ps_proj, \
                     tc.tile_pool(name="ps_tr2", bufs=2, space="PSUM") as ps_tr:
                    pending = None  # (t, ht) awaiting PE transposes
                    for t in range(NT):
                        ps = ps_proj.tile([128, C], f32, tag="mm")
                        for off in (0, 512):
                            for cp in range(4):
                                nc.tensor.matmul(
                                    ps[:, off:off + 512],
                                    lhsT=attT8[:, 2 * cp:2 * cp + 2,
                                               128 * t:128 * (t + 1)],
                                    rhs=wp8[:, 2 * cp:2 * cp + 2,
                                            off:off + 512],
                                    start=(cp == 0), stop=(cp == 3),
                                    perf_mode=DR)
                        if pending is not None:
                            ln_transpose(pending[0], pending[1], ps_tr)
                        nc.vector.tensor_tensor(x_sb[t][:], ps[:], x_sb[t][:],
                                                OP.add)
                        if flags["biasp_nz"]:
                            nc.vector.tensor_tensor(x_sb[t][:], x_sb[t][:],
                                                    bp_sb[:], OP.add)
                        pending = (t, ln_tile(t))
                    ln_transpose(pending[0], pending[1], ps_tr)

                # ======================= phase 6: FFN =======================
                f1 = [f1pool.tile([128, NFF, 512], f8, name=f"f1_{i}")
                      for i in range(2)]
                with tc.tile_pool(name="ps_f1", bufs=2, space="PSUM") as ps_f1, \
                     tc.tile_pool(name="ps_y2", bufs=1, space="PSUM") as ps_y2:

                    def ffn1_half(half):
                        for f in range(NFF):
                            ps = ps_f1.tile([128, 512], f32, tag="mm")
                            for cp in range(4):
                                nc.tensor.matmul(
                                    ps[:],
                                    lhsT=w18[:, 2 * cp:2 * cp + 2,
                                             128 * f:128 * (f + 1)],
                                    rhs=hT[half][:, 2 * cp:2 * cp + 2, :],
                                    start=(cp == 0), stop=(cp == 3),
                                    perf_mode=DR)
                            if flags["b1_nz"]:
                                nc.scalar.activation(f1[half][:, f, :], ps[:],
                                                     AF.Relu,
                                                     bias=b1e_sb[:, f:f + 1])
                            elif f % 2 == 0:
                                nc.scalar.activation(f1[half][:, f, :], ps[:],
                                                     AF.Relu)
                            else:
                                nc.vector.tensor_scalar_max(f1[half][:, f, :],
                                                            ps[:], 0.0)

                    def ffn2_group(tg):
                        for off in (0, 512):
                            trange = range(4 * tg, 4 * tg + 4)
                            y2 = {t: ps_y2.tile([128, 512], f32,
                                                tag=f"y2_{t % 4}",
                                                name=f"y2_{off}_{t}")
                                  for t in trange}
                            for fp in range(NFF // 2):
                                for t in trange:
                                    tl = t % 4
                                    nc.tensor.matmul(
                                        y2[t][:],
                                        lhsT=f1[tg][:, 2 * fp:2 * fp + 2,
                                                    128 * tl:128 * (tl + 1)],
                                        rhs=w28[:, 2 * fp:2 * fp + 2,
                                                off:off + 512],
                                        start=(fp == 0), stop=(fp == 15),
                                        perf_mode=DR)
                            for t in trange:
                                nc.vector.tensor_tensor(
                                    x_sb[t][:, off:off + 512], y2[t][:],
                                    x_sb[t][:, off:off + 512], OP.add)
                                if flags["bias2_nz"]:
                                    nc.vector.tensor_tensor(
                                        x_sb[t][:, off:off + 512],
                                        x_sb[t][:, off:off + 512],
                                        b2_sb[:, off:off + 512], OP.add)
                                if off == 512:
                                    nc.sync.dma_start(
                                        outd[128 * t:128 * (t + 1), 0:512],
                                        x_sb[t][:, 0:512])
                                    nc.sync.dma_start(
                                        outd[128 * t:128 * (t + 1), 512:1024],
                                        x_sb[t][:, 512:1024])

                    ffn1_half(0)   # f1[0] = all f for t 0..511
                    ffn2_group(0)  # FFN2 on t 0..511 chases FFN1 half 1
                    ffn1_half(1)
                    ffn2_group(1)
    nc.compile()
    return nc


def _get_nc(flags):
    key = tuple(sorted(flags.items()))
    if key not in _CACHE:
        _CACHE[key] = build(flags)
    return _CACHE[key]


# ----------------------------------------------------------------------------
# public entry point
# ----------------------------------------------------------------------------

def kernel(**inputs):
    from concourse import bass_utils
    x = np.asarray(inputs["x"], np.float32)
    d, flags = _prep(inputs)
    nc = _get_nc(flags)
    in_maps = []
    for b in range(B):
        m = dict(d)
        m["x"] = np.ascontiguousarray(x[b])
        in_maps.append(m)
    res = bass_utils.run_bass_kernel_spmd(nc, in_maps, core_ids=list(range(B)))
    out = np.stack([r["out"] for r in res.results]).astype(np.float32)
    return out


# revision 33
# speedup vs baseline: 1.5402x; 1.1946x over previous
"""Trainium2 Bass kernel for a dense transformer block (B=8, T=1024, C=1024, H=16, FF=4096).

Sharding: data-parallel over batch - one batch element per NeuronCore (8 cores),
no collectives.

Speed levers vs the bf16 baseline (631 us):
  * fp8e4m3 DoubleRow matmuls for QKV, proj, FFN1, FFN2 and the attention AV
    contraction: each PE instruction contracts 256 rows instead of 128 at the
    same column rate, halving PE time for the big GEMMs. The fake-quantized
    weights are integers k*2^-8 with |k|<=8 - EXACT in fp8e4m3; only
    activations pick up ~2% rounding noise (end-to-end rel err ~9.6e-3,
    gate 2e-2).
  * Engine balance: during attention ACT runs only Exp (no act-table swaps);
    DVE does LN stats, transpose evacuation, softmax normalize, residual
    adds; GpSimd does causal masks, zero-fills and r broadcasts.
  * Pipelining: weight DMAs split into chunks across queues; QKV starts on
    the first t-half of LN1; S/exp and AV interleave per s-tile pair; LN2
    transposes stagger between proj matmuls; FFN1 runs half-major so FFN2
    chases it; outputs stream out per t-group.

Layouts (per core):
  hT    [128, 8, 512] fp8 x2 : hT[p, i, t] = h[t, 128*i + p] (t-halves; LN1 then LN2)
  w*8   [128, K/128, M] fp8  : w[p, i, m] = W[m, 128*i + p] (DoubleRow pairs = dim1)
  qT/kT [128, 1024] fp8 per head-pair (rows 64e+d)
  v8    [128, 8, 16, 128] fp8: cols 0-63 ones (sums -> avp rows 0-63), 64-127 = v
  exp8  [128, 8, 2, 1024] fp8 per pair: exp8[s, j, e, t] = exp(S[t, 128j+s]) unnorm
  attT8 [128, 8, 1024] fp8   : attT8[64e+d, p8, t] = normalized att
  f1    [128, 32, 512] fp8 x2: f1[p, f, t] = relu(ffn1)[t, 128f+p] (t-halves)
"""

import os
import numpy as np
import ml_dtypes

DEBUG = bool(int(os.environ.get("BASSDBG", "0")))

B, T, C, H = 8, 1024, 1024, 16
HS = C // H          # 64
FF = 4 * C           # 4096
EPS = 1e-5
NT = T // 128        # 8 t-tiles
NCI = C // 128       # 8 c-tiles
NFF = FF // 128      # 32 ff-tiles
VW = 128             # per-head stride in v8: cols 0-63 ones (sums on avp row 0
                     # for partition_broadcast), v in cols 64-127 (partition
                     # ranges of 64 must start at 0 or 64)
SM_SCALE = 1.0 / 32.0  # C ** -0.5

_CACHE = {}

npf8 = ml_dtypes.float8_e4m3  # TRN fp8e4 (max 240)


# ----------------------------------------------------------------------------
# host-side math (exact reference semantics)
# ----------------------------------------------------------------------------

def _quant_weight(W, e, b):
    W = np.asarray(W, np.float32)
    e = np.asarray(e, np.float32)
    b = np.asarray(b, np.float32)
    b_rel = np.maximum(b, 0.0)
    mn = np.where(b_rel > 0, -(2.0 ** (b_rel - 1)), 0.0)
    mx = np.where(b_rel > 0, 2.0 ** (b_rel - 1) - 1.0, 0.0)
    qw = np.clip((2.0 ** (-e)) * W, mn, mx)
    w = np.round(qw)  # round-half-even, same as jnp.round
    return ((2.0 ** e) * w).astype(np.float32)


def _dr_layout(WT):
    """[K, M] -> [128, K//128, M] fp8 DoubleRow layout: out[p, i, m] = WT[128i+p, m]."""
    K, M = WT.shape
    return np.ascontiguousarray(
        WT.reshape(K // 128, 128, M).transpose(1, 0, 2)).astype(npf8)


def _prep(inputs):
    f32 = np.float32
    g1 = np.asarray(inputs["g1"], f32)
    be1 = np.asarray(inputs["be1"], f32)
    g2 = np.asarray(inputs["g2"], f32)
    be2 = np.asarray(inputs["be2"], f32)

    Wq = _quant_weight(inputs["Wq"], inputs["eq"], inputs["bq"])  # [H,HS,C]
    Wk = _quant_weight(inputs["Wk"], inputs["ek"], inputs["bk"])
    Wv = _quant_weight(inputs["Wv"], inputs["ev"], inputs["bv"])
    Wp = _quant_weight(inputs["Wp"], inputs["ep"], inputs["bp"])  # [C,C]
    W1 = _quant_weight(inputs["W1"], inputs["e1"], inputs["b1"])  # [FF,C]
    W2 = _quant_weight(inputs["W2"], inputs["e2"], inputs["b2"])  # [C,FF]

    # fold LN gains into the adjacent weights (identity when g == 1, so the
    # fp8 cast of the quantized weights stays exact in that case)
    Wqf = (Wq * g1[None, None, :]).reshape(H * HS, C)
    Wkf = (Wk * g1[None, None, :]).reshape(H * HS, C)
    Wvf = (Wv * g1[None, None, :]).reshape(H * HS, C)
    W1f = W1 * g2[None, :]

    d = {
        "wq8": _dr_layout(Wqf.T), "wk8": _dr_layout(Wkf.T), "wv8": _dr_layout(Wvf.T),
        "wp8": _dr_layout(np.ascontiguousarray(Wp.T)),
        "w18": _dr_layout(np.ascontiguousarray(W1f.T)),
        "w28": _dr_layout(np.ascontiguousarray(W2.T)),
    }
    # biases from LN betas routed through the projections
    qb = (Wqf @ be1).astype(f32)   # [H*HS]
    kb = (Wkf @ be1).astype(f32)
    vb = (Wvf @ be1).astype(f32)
    b1e = (np.asarray(inputs["bias1"], f32) + W1 @ be2).astype(f32)  # [FF]
    d["qb"] = np.ascontiguousarray(qb.reshape(8, 128).T)   # [128, 8]
    d["kb"] = np.ascontiguousarray(kb.reshape(8, 128).T)
    d["b1e"] = np.ascontiguousarray(b1e.reshape(NFF, 128).T)  # [128, 32]
    vb_pad = np.zeros(H * VW, f32)
    for h in range(H):
        vb_pad[h * VW + HS: h * VW + 2 * HS] = vb[h * HS:(h + 1) * HS]
    d["vbpad"] = vb_pad
    d["biasp"] = np.asarray(inputs["biasp"], f32)
    d["bias2"] = np.asarray(inputs["bias2"], f32)
    # causal 0/1 mask for diagonal S^T blocks (keep t_local >= s_local),
    # duplicated side by side so one op covers both heads of a pair
    mask = (np.arange(128)[None, :] >= np.arange(128)[:, None]).astype(npf8)
    d["mask8"] = np.ascontiguousarray(np.tile(mask, (1, 2)))   # [128, 256]
    flags = {
        "qb_nz": bool(np.any(qb != 0)),
        "kb_nz": bool(np.any(kb != 0)),
        "vb_nz": bool(np.any(vb != 0)),
        "b1_nz": bool(np.any(b1e != 0)),
        "biasp_nz": bool(np.any(d["biasp"] != 0)),
        "bias2_nz": bool(np.any(d["bias2"] != 0)),
    }
    return d, flags


# ----------------------------------------------------------------------------
# device kernel
# ----------------------------------------------------------------------------

def build(flags):
    import concourse.bass as bass
    import concourse.tile as tile
    from concourse import bacc, mybir

    f32 = mybir.dt.float32
    bf16 = mybir.dt.bfloat16
    f8 = mybir.dt.float8e4
    AF = mybir.ActivationFunctionType
    OP = mybir.AluOpType
    DR = mybir.MatmulPerfMode.DoubleRow

    nc = bacc.Bacc("TRN2", target_bir_lowering=False)

    xd = nc.dram_tensor("x", [T, C], f32, kind="ExternalInput")
    wq8d = nc.dram_tensor("wq8", [128, NCI, C], f8, kind="ExternalInput")
    wk8d = nc.dram_tensor("wk8", [128, NCI, C], f8, kind="ExternalInput")
    wv8d = nc.dram_tensor("wv8", [128, NCI, C], f8, kind="ExternalInput")
    wp8d = nc.dram_tensor("wp8", [128, NCI, C], f8, kind="ExternalInput")
    w18d = nc.dram_tensor("w18", [128, NCI, FF], f8, kind="ExternalInput")
    w28d = nc.dram_tensor("w28", [128, NFF, C], f8, kind="ExternalInput")
    qbd = nc.dram_tensor("qb", [128, 8], f32, kind="ExternalInput")
    kbd = nc.dram_tensor("kb", [128, 8], f32, kind="ExternalInput")
    b1ed = nc.dram_tensor("b1e", [128, NFF], f32, kind="ExternalInput")
    mask8d = nc.dram_tensor("mask8", [128, 256], f8, kind="ExternalInput")
    vbpd = nc.dram_tensor("vbpad", [H * VW], f32, kind="ExternalInput")
    biaspd = nc.dram_tensor("biasp", [C], f32, kind="ExternalInput")
    bias2d = nc.dram_tensor("bias2", [C], f32, kind="ExternalInput")
    outd = nc.dram_tensor("out", [T, C], f32, kind="ExternalOutput")

    def bcast_dram_row(vec_ap, n):
        return bass.AP(tensor=vec_ap.tensor, offset=vec_ap.offset,
                       ap=[[0, 128], [1, n]])

    def dma_chunked(dst, src, dim1, n=1):
        # n DMAs spread across queues (high n steals SBUF bandwidth from PE)
        step = dim1 // n
        for i in range(n):
            nc.sync.dma_start(dst[:, step * i:step * (i + 1), :],
                              src[:, step * i:step * (i + 1), :])

    with tile.TileContext(nc) as tc, \
         tc.tile_pool(name="consts", bufs=1) as consts, \
         tc.tile_pool(name="xpool", bufs=1) as xpool, \
         tc.tile_pool(name="hpool", bufs=1) as hpool, \
         tc.tile_pool(name="ln_tmp", bufs=3) as ln_tmp:

        # ---- constants ----
        from concourse.masks import make_identity
        ident8 = consts.tile([128, 128], f8, name="ident8")
        make_identity(nc, ident8[:])
        mask8_sb = consts.tile([128, 2, 128], f8, name="mask8_sb")
        nc.sync.dma_start(mask8_sb[:].rearrange("p e t -> p (e t)"),
                          mask8d[:, :])
        eps_sb = consts.tile([128, 1], f32, name="eps_sb")
        nc.vector.memset(eps_sb[:], EPS)
        if flags["qb_nz"]:
            qb_sb = consts.tile([128, 8], f32, name="qb_sb")
            nc.sync.dma_start(qb_sb[:], qbd[:, :])
        if flags["kb_nz"]:
            kb_sb = consts.tile([128, 8], f32, name="kb_sb")
            nc.sync.dma_start(kb_sb[:], kbd[:, :])
        if flags["b1_nz"]:
            b1e_sb = consts.tile([128, NFF], f32, name="b1e_sb")
            nc.sync.dma_start(b1e_sb[:], b1ed[:, :])
        if flags["vb_nz"]:
            vb_sb = consts.tile([128, H * VW], f32, name="vb_sb")
            nc.sync.dma_start(vb_sb[:], bcast_dram_row(vbpd[:], H * VW))
        if flags["biasp_nz"]:
            bp_sb = consts.tile([128, C], f32, name="bp_sb")
            nc.sync.dma_start(bp_sb[:], bcast_dram_row(biaspd[:], C))
        if flags["bias2_nz"]:
            b2_sb = consts.tile([128, C], f32, name="b2_sb")
            nc.sync.dma_start(b2_sb[:], bcast_dram_row(bias2d[:], C))

        # ---- x tiles (persist; become xnew, then the output) ----
        x_sb = []
        for t in range(NT):
            xt = xpool.tile([128, C], f32, name=f"x{t}")
            nc.sync.dma_start(xt[:, 0:512], xd[128 * t:128 * (t + 1), 0:512])
            nc.sync.dma_start(xt[:, 512:1024],
                              xd[128 * t:128 * (t + 1), 512:1024])
            x_sb.append(xt)

        # hT in two t-halves so consumers can start after 4 t-tiles
        hT = [hpool.tile([128, NCI, 512], f8, name=f"hT{i}") for i in range(2)]

        def ln_tile(t):
            """LN stats + normalize for x tile t -> fp8 h tile (returns it)."""
            xt = x_sb[t]
            stats = ln_tmp.tile([128, 2, 6], f32, tag="lnstats")
            nc.vector.bn_stats(stats[:, 0, :], xt[:, 0:512])
            nc.vector.bn_stats(stats[:, 1, :], xt[:, 512:1024])
            mv = ln_tmp.tile([128, 2], f32, tag="lnmv")
            nc.vector.bn_aggr(mv[:], stats[:])
            rstd = ln_tmp.tile([128, 1], f32, tag="lnrstd")
            nc.scalar.activation(rstd[:], mv[:, 1:2], AF.Sqrt, bias=eps_sb[:])
            nc.vector.reciprocal(rstd[:], rstd[:])
            nmr = ln_tmp.tile([128, 1], f32, tag="lnnmr")
            nc.vector.tensor_scalar(nmr[:], mv[:, 0:1], rstd[:], -1.0,
                                    OP.mult, OP.mult)
            ht = ln_tmp.tile([128, C], f8, tag="lnh")
            nc.scalar.activation(ht[:], xt[:], AF.Identity,
                                 bias=nmr[:], scale=rstd[:])
            return ht

        def ln_transpose(t, ht, ps_tr):
            """PE-transpose h tile t into hT[t // 4] (batched evacuation)."""
            ptr = ps_tr.tile([128, NCI, 128, 2], f8, tag="tr")
            for c in range(NCI):
                nc.tensor.transpose(ptr[:, c, :, 0],
                                    ht[:, 128 * c:128 * (c + 1)], ident8[:])
            half, tl = divmod(t, 4)
            nc.vector.tensor_copy(hT[half][:, :, 128 * tl:128 * (tl + 1)],
                                  ptr[:, :, :, 0])

        with tc.tile_pool(name="wpp", bufs=1) as wpp, \
             tc.tile_pool(name="w1p", bufs=1) as w1p, \
             tc.tile_pool(name="att", bufs=1) as att, \
             tc.tile_pool(name="qkv", bufs=1) as qkv:
            attT8 = att.tile([128, NCI, T], f8, name="attT8")
            qT_sb = [qkv.tile([128, T], f8, name=f"qT{p}") for p in range(8)]
            kT_sb = [qkv.tile([128, T], f8, name=f"kT{p}") for p in range(8)]
            v8 = qkv.tile([128, NT, H, VW], f8, name="v8")
            # ones in cols 0..63 -> avp rows 0..63 all hold the softmax sums
            nc.gpsimd.memset(v8[:, :, :, 0:HS], 1.0)

            with tc.tile_pool(name="wqkv", bufs=1) as wqkv:
                wq8 = wqkv.tile([128, NCI, C], f8, name="wq8")
                dma_chunked(wq8, wq8d, NCI)
                wk8 = wqkv.tile([128, NCI, C], f8, name="wk8")
                dma_chunked(wk8, wk8d, NCI)
                wv8 = wqkv.tile([128, NCI, C], f8, name="wv8")
                dma_chunked(wv8, wv8d, NCI)

                # ====================== phase 1: LN1 ========================
                with tc.tile_pool(name="ps_tr1", bufs=2, space="PSUM") as ps_tr:
                    for t in range(NT):
                        ln_transpose(t, ln_tile(t), ps_tr)

                # ====================== phase 2: QKV ========================
                with tc.tile_pool(name="ps_qkv", bufs=2, space="PSUM") as ps_qkv:
                    for (w8, dst, bias_nm, b_nz) in (
                            (wq8, qT_sb, "qb", flags["qb_nz"]),
                            (wk8, kT_sb, "kb", flags["kb_nz"])):
                        for p in range(8):
                            ps = ps_qkv.tile([128, T], f32, tag="mm")
                            for half in (0, 1):
                                for cp in range(4):
                                    nc.tensor.matmul(
                                        ps[:, 512 * half:512 * half + 512],
                                        lhsT=w8[:, 2 * cp:2 * cp + 2,
                                                128 * p:128 * (p + 1)],
                                        rhs=hT[half][:, 2 * cp:2 * cp + 2, :],
                                        start=(cp == 0), stop=(cp == 3),
                                        perf_mode=DR)
                            if b_nz:
                                bias_sb = qb_sb if bias_nm == "qb" else kb_sb
                                nc.scalar.activation(dst[p][:], ps[:],
                                                     AF.Identity,
                                                     bias=bias_sb[:, p:p + 1])
                            else:
                                nc.scalar.activation(dst[p][:], ps[:],
                                                     AF.Identity)
                    # v: [t-part, head-major d]
                    for t in range(NT):
                        half, tl = divmod(t, 4)
                        ps = ps_qkv.tile([128, T], f32, tag="mm")
                        for off in (0, 512):
                            for cp in range(4):
                                nc.tensor.matmul(
                                    ps[:, off:off + 512],
                                    lhsT=hT[half][:, 2 * cp:2 * cp + 2,
                                                  128 * tl:128 * (tl + 1)],
                                    rhs=wv8[:, 2 * cp:2 * cp + 2,
                                            off:off + 512],
                                    start=(cp == 0), stop=(cp == 3),
                                    perf_mode=DR)
                        ps3 = ps[:].rearrange("p (h d) -> p h d", d=HS)
                        if flags["vb_nz"]:
                            vb3 = vb_sb[:].rearrange("p (h w) -> p h w", w=VW)
                            nc.vector.tensor_tensor(v8[:, t, :, HS:2 * HS],
                                                    ps3,
                                                    vb3[:, :, HS:2 * HS],
                                                    OP.add)
                        else:
                            nc.vector.tensor_copy(v8[:, t, :, HS:2 * HS], ps3)
            # wqkv pool closed - wq/wk/wv freed before attention

            # ===================== phase 3: attention =======================
            # prefetch Wp and W1 while attention runs
            wp8 = wpp.tile([128, NCI, C], f8, name="wp8")
            dma_chunked(wp8, wp8d, NCI)
            w18 = w1p.tile([128, NCI, FF], f8, name="w18")
            dma_chunked(w18, w18d, NCI)

            with tc.tile_pool(name="exp_pool", bufs=2) as exp_pool, \
                 tc.tile_pool(name="r_pool", bufs=2) as r_pool, \
                 tc.tile_pool(name="rr_pool", bufs=2) as rr_pool, \
                 tc.tile_pool(name="ps_st", bufs=2, space="PSUM") as ps_st, \
                 tc.tile_pool(name="ps_av", bufs=2, space="PSUM") as ps_av:
                for p8 in range(8):
                    ex = exp_pool.tile([128, NT, 2, T], f8, tag="exp",
                                       name=f"exp{p8}")
                    # zero-fill the non-causal 128-col blocks of odd s-tiles
                    # (DoubleRow pairs s-tiles (2a, 2a+1); the shared window
                    # starts at t=256a)
                    for a in range(4):
                        for e in (0, 1):
                            nc.gpsimd.memset(
                                ex[:, 2 * a + 1, e, 256 * a:256 * a + 128],
                                0.0)
                    for j in range(NT):
                        W = T - 128 * j
                        for e in (0, 1):
                            po = 64 * e
                            st = ps_st.tile([128, T], f32, tag="st",
                                            name=f"st{p8}_{j}_{e}")
                            for off in range(0, W, 512):
                                w = min(512, W - off)
                                nc.tensor.matmul(
                                    st[:, off:off + w],
                                    lhsT=kT_sb[p8][po:po + 64,
                                                   128 * j:128 * (j + 1)],
                                    rhs=qT_sb[p8][po:po + 64,
                                                  128 * j + off:
                                                  128 * j + off + w],
                                    start=True, stop=True)
                            nc.scalar.activation(
                                ex[:, j, e, 128 * j:T], st[:, 0:W],
                                AF.Exp, scale=SM_SCALE)
                        # causal mask on both heads' diagonal blocks
                        nc.vector.tensor_tensor(
                            ex[:, j, 0:2, 128 * j:128 * (j + 1)],
                            ex[:, j, 0:2, 128 * j:128 * (j + 1)],
                            mask8_sb[:], OP.mult)
                    for e in (0, 1):
                        h = 2 * p8 + e
                        avp_e = ps_av.tile([VW, T], f32, tag="av",
                                           name=f"av{h}")
                        for off in (0, 512):
                            aa = [q for q in range(4) if 256 * q < off + 512]
                            for a in aa:
                                lo = max(off, 256 * a)
                                nc.tensor.matmul(
                                    avp_e[0:VW, lo:off + 512],
                                    lhsT=v8[:, 2 * a:2 * a + 2, h, 0:VW],
                                    rhs=ex[:, 2 * a:2 * a + 2, e,
                                           lo:off + 512],
                                    start=(a == aa[0]), stop=(a == aa[-1]),
                                    perf_mode=DR)
                        # r = 1/sums (avp row 0); attT = att_unnorm * r
                        # (fp8). reciprocal_approx_fast mis-executes on
                        # 1-partition APs: recip the whole avp, use row 0.
                        if True:
                            r_sb = r_pool.tile([VW, T], f32, tag="r",
                                               name=f"r{2 * p8 + e}")
                            nc.vector.reciprocal_approx_fast(r_sb[:],
                                                             avp_e[0:VW, :])
                            rr = rr_pool.tile([128, T], f32, tag="rr",
                                              name=f"rr{2 * p8 + e}")
                            nc.gpsimd.partition_broadcast(rr[:], r_sb[0:1, :],
                                                          channels=128)
                            nc.vector.tensor_tensor(
                                attT8[64 * e:64 * e + 64, p8, :],
                                avp_e[HS:2 * HS, :], rr[0:64, :], OP.mult)
        # qkv + attention pools closed here

            # ============== phase 4+5: proj + LN2 (staggered) ===============
            with tc.tile_pool(name="w2p", bufs=1) as w2p, \
                 tc.tile_pool(name="f1pool", bufs=1) as f1pool:
                w28 = w2p.tile([128, NFF, C], f8, name="w28")
                dma_chunked(w28, w28d, NFF, 1)

                with tc.tile_pool(name="ps_proj", bufs=2, space="PSUM") as ps_proj, \
                     tc.tile_pool(name="ps_tr2", bufs=2, space="PSUM") as ps_tr:
                    pending = None  # (t, ht) awaiting PE transposes
                    for t in range(NT):
                        ps = ps_proj.tile([128, C], f32, tag="mm")
                        for off in (0, 512):
                            for cp in range(4):
                                nc.tensor.matmul(
                                    ps[:, off:off + 512],
                                    lhsT=attT8[:, 2 * cp:2 * cp + 2,
                                               128 * t:128 * (t + 1)],
                                    rhs=wp8[:, 2 * cp:2 * cp + 2,
                                            off:off + 512],
                                    start=(cp == 0), stop=(cp == 3),
                                    perf_mode=DR)
                        if pending is not None:
                            ln_transpose(pending[0], pending[1], ps_tr)
                        nc.vector.tensor_tensor(x_sb[t][:], ps[:], x_sb[t][:],
                                                OP.add)
                        if flags["biasp_nz"]:
                            nc.vector.tensor_tensor(x_sb[t][:], x_sb[t][:],
                                                    bp_sb[:], OP.add)
                        pending = (t, ln_tile(t))
                    ln_transpose(pending[0], pending[1], ps_tr)

                # ======================= phase 6: FFN =======================
                f1 = [f1pool.tile([128, NFF, 512], f8, name=f"f1_{i}")
                      for i in range(2)]
                with tc.tile_pool(name="ps_f1", bufs=2, space="PSUM") as ps_f1, \
                     tc.tile_pool(name="ps_y2", bufs=1, space="PSUM") as ps_y2:

                    def ffn1_half(half):
                        for f in range(NFF):
                            ps = ps_f1.tile([128, 512], f32, tag="mm")
                            for cp in range(4):
                                nc.tensor.matmul(
                                    ps[:],
                                    lhsT=w18[:, 2 * cp:2 * cp + 2,
                                             128 * f:128 * (f + 1)],
                                    rhs=hT[half][:, 2 * cp:2 * cp + 2, :],
                                    start=(cp == 0), stop=(cp == 3),
                                    perf_mode=DR)
                            if flags["b1_nz"]:
                                nc.scalar.activation(f1[half][:, f, :], ps[:],
                                                     AF.Relu,
                                                     bias=b1e_sb[:, f:f + 1])
                            elif f % 2 == 0:
                                nc.scalar.activation(f1[half][:, f, :], ps[:],
                                                     AF.Relu)
                            else:
                                nc.vector.tensor_scalar_max(f1[half][:, f, :],
                                                            ps[:], 0.0)

                    def ffn2_group(tg):
                        for off in (0, 512):
                            trange = range(4 * tg, 4 * tg + 4)
                            y2 = {t: ps_y2.tile([128, 512], f32,
                                                tag=f"y2_{t % 4}",
                                                name=f"y2_{off}_{t}")
                                  for t in trange}
                            for fp in range(NFF // 2):
                                for t in trange:
                                    tl = t % 4
                                    nc.tensor.matmul(
                                        y2[t][:],
                                        lhsT=f1[tg][:, 2 * fp:2 * fp + 2,
                                                    128 * tl:128 * (tl + 1)],
                                        rhs=w28[:, 2 * fp:2 * fp + 2,
                                                off:off + 512],
                                        start=(fp == 0), stop=(fp == 15),
                                        perf_mode=DR)
                            for t in trange:
                                nc.vector.tensor_tensor(
                                    x_sb[t][:, off:off + 512], y2[t][:],
                                    x_sb[t][:, off:off + 512], OP.add)
                                if flags["bias2_nz"]:
                                    nc.vector.tensor_tensor(
                                        x_sb[t][:, off:off + 512],
                                        x_sb[t][:, off:off + 512],
                                        b2_sb[:, off:off + 512], OP.add)
                                if off == 512:
                                    nc.sync.dma_start(
                                        outd[128 * t:128 * (t + 1), 0:512],
                                        x_sb[t][:, 0:512])
                                    nc.sync.dma_start(
                                        outd[128 * t:128 * (t + 1), 512:1024],
                                        x_sb[t][:, 512:1024])

                    ffn1_half(0)   # f1[0] = all f for t 0..511
                    ffn2_group(0)  # FFN2 on t 0..511 chases FFN1 half 1
                    ffn1_half(1)
                    ffn2_group(1)
    nc.compile()
    return nc


def _get_nc(flags):
    key = tuple(sorted(flags.items()))
    if key not in _CACHE:
        _CACHE[key] = build(flags)
    return _CACHE[key]


# ----------------------------------------------------------------------------
# public entry point
# ----------------------------------------------------------------------------

def kernel(**inputs):
    from concourse import bass_utils
    x = np.asarray(inputs["x"], np.float32)
    d, flags = _prep(inputs)
    nc = _get_nc(flags)
    in_maps = []
    for b in range(B):
        m = dict(d)
        m["x"] = np.ascontiguousarray(x[b])
        in_maps.append(m)
    res = bass_utils.run_bass_kernel_spmd(nc, in_maps, core_ids=list(range(B)))
    out = np.stack([r["out"] for r in res.results]).astype(np.float32)
    return out


# revision 35
# speedup vs baseline: 1.5711x; 1.0200x over previous
"""Trainium2 Bass kernel for a dense transformer block (B=8, T=1024, C=1024, H=16, FF=4096).

Sharding: data-parallel over batch - one batch element per NeuronCore (8 cores),
no collectives.

Speed levers vs the bf16 baseline (631 us):
  * fp8e4m3 DoubleRow matmuls for QKV, proj, FFN1, FFN2 and the attention AV
    contraction: each PE instruction contracts 256 rows instead of 128 at the
    same column rate, halving PE time for the big GEMMs. The fake-quantized
    weights are integers k*2^-8 with |k|<=8 - EXACT in fp8e4m3; only
    activations pick up ~2% rounding noise (end-to-end rel err ~9.6e-3,
    gate 2e-2).
  * Engine balance: during attention ACT runs only Exp (no act-table swaps);
    DVE does LN stats, transpose evacuation, softmax normalize, residual
    adds; GpSimd does causal masks, zero-fills and r broadcasts.
  * Pipelining: weight DMAs split into chunks across queues; QKV starts on
    the first t-half of LN1; S/exp and AV interleave per s-tile pair; LN2
    transposes stagger between proj matmuls; FFN1 runs half-major so FFN2
    chases it; outputs stream out per t-group.

Layouts (per core):
  hT    [128, 8, 512] fp8 x2 : hT[p, i, t] = h[t, 128*i + p] (t-halves; LN1 then LN2)
  w*8   [128, K/128, M] fp8  : w[p, i, m] = W[m, 128*i + p] (DoubleRow pairs = dim1)
  qT/kT [128, 1024] fp8 per head-pair (rows 64e+d)
  v8    [128, 8, 16, 128] fp8: cols 0-63 ones (sums -> avp rows 0-63), 64-127 = v
  exp8  [128, 8, 2, 1024] fp8 per pair: exp8[s, j, e, t] = exp(S[t, 128j+s]) unnorm
  attT8 [128, 8, 1024] fp8   : attT8[64e+d, p8, t] = normalized att
  f1    [128, 32, 512] fp8 x2: f1[p, f, t] = relu(ffn1)[t, 128f+p] (t-halves)
"""

import os
import numpy as np
import ml_dtypes

DEBUG = bool(int(os.environ.get("BASSDBG", "0")))

B, T, C, H = 8, 1024, 1024, 16
HS = C // H          # 64
FF = 4 * C           # 4096
EPS = 1e-5
NT = T // 128        # 8 t-tiles
NCI = C // 128       # 8 c-tiles
NFF = FF // 128      # 32 ff-tiles
VW = 128             # per-head stride in v8: cols 0-63 ones (sums on avp row 0
                     # for partition_broadcast), v in cols 64-127 (partition
                     # ranges of 64 must start at 0 or 64)
SM_SCALE = 1.0 / 32.0  # C ** -0.5

_CACHE = {}

npf8 = ml_dtypes.float8_e4m3  # TRN fp8e4 (max 240)


# ----------------------------------------------------------------------------
# host-side math (exact reference semantics)
# ----------------------------------------------------------------------------

def _quant_weight(W, e, b):
    W = np.asarray(W, np.float32)
    e = np.asarray(e, np.float32)
    b = np.asarray(b, np.float32)
    b_rel = np.maximum(b, 0.0)
    mn = np.where(b_rel > 0, -(2.0 ** (b_rel - 1)), 0.0)
    mx = np.where(b_rel > 0, 2.0 ** (b_rel - 1) - 1.0, 0.0)
    qw = np.clip((2.0 ** (-e)) * W, mn, mx)
    w = np.round(qw)  # round-half-even, same as jnp.round
    return ((2.0 ** e) * w).astype(np.float32)


def _dr_layout(WT):
    """[K, M] -> [128, K//128, M] fp8 DoubleRow layout: out[p, i, m] = WT[128i+p, m]."""
    K, M = WT.shape
    return np.ascontiguousarray(
        WT.reshape(K // 128, 128, M).transpose(1, 0, 2)).astype(npf8)


def _prep(inputs):
    f32 = np.float32
    g1 = np.asarray(inputs["g1"], f32)
    be1 = np.asarray(inputs["be1"], f32)
    g2 = np.asarray(inputs["g2"], f32)
    be2 = np.asarray(inputs["be2"], f32)

    Wq = _quant_weight(inputs["Wq"], inputs["eq"], inputs["bq"])  # [H,HS,C]
    Wk = _quant_weight(inputs["Wk"], inputs["ek"], inputs["bk"])
    Wv = _quant_weight(inputs["Wv"], inputs["ev"], inputs["bv"])
    Wp = _quant_weight(inputs["Wp"], inputs["ep"], inputs["bp"])  # [C,C]
    W1 = _quant_weight(inputs["W1"], inputs["e1"], inputs["b1"])  # [FF,C]
    W2 = _quant_weight(inputs["W2"], inputs["e2"], inputs["b2"])  # [C,FF]

    # fold LN gains into the adjacent weights (identity when g == 1, so the
    # fp8 cast of the quantized weights stays exact in that case)
    Wqf = (Wq * g1[None, None, :]).reshape(H * HS, C)
    Wkf = (Wk * g1[None, None, :]).reshape(H * HS, C)
    Wvf = (Wv * g1[None, None, :]).reshape(H * HS, C)
    W1f = W1 * g2[None, :]

    d = {
        "wq8": _dr_layout(Wqf.T), "wk8": _dr_layout(Wkf.T), "wv8": _dr_layout(Wvf.T),
        "wp8": _dr_layout(np.ascontiguousarray(Wp.T)),
        "w18": _dr_layout(np.ascontiguousarray(W1f.T)),
        "w28": _dr_layout(np.ascontiguousarray(W2.T)),
    }
    # biases from LN betas routed through the projections
    qb = (Wqf @ be1).astype(f32)   # [H*HS]
    kb = (Wkf @ be1).astype(f32)
    vb = (Wvf @ be1).astype(f32)
    b1e = (np.asarray(inputs["bias1"], f32) + W1 @ be2).astype(f32)  # [FF]
    d["qb"] = np.ascontiguousarray(qb.reshape(8, 128).T)   # [128, 8]
    d["kb"] = np.ascontiguousarray(kb.reshape(8, 128).T)
    d["b1e"] = np.ascontiguousarray(b1e.reshape(NFF, 128).T)  # [128, 32]
    vb_pad = np.zeros(H * VW, f32)
    for h in range(H):
        vb_pad[h * VW + HS: h * VW + 2 * HS] = vb[h * HS:(h + 1) * HS]
    d["vbpad"] = vb_pad
    d["biasp"] = np.asarray(inputs["biasp"], f32)
    d["bias2"] = np.asarray(inputs["bias2"], f32)
    # additive causal mask for diagonal S^T blocks: 0 where t_local >=
    # s_local else -1e4 (exp(-1e4/32) == 0); applied by a PE matmul
    # (identity stationary) accumulating onto the diagonal PSUM block
    keep = np.arange(128)[None, :] >= np.arange(128)[:, None]
    d["maskneg"] = np.where(keep, 0.0, -1.0e4).astype(ml_dtypes.bfloat16)
    flags = {
        "qb_nz": bool(np.any(qb != 0)),
        "kb_nz": bool(np.any(kb != 0)),
        "vb_nz": bool(np.any(vb != 0)),
        "b1_nz": bool(np.any(b1e != 0)),
        "biasp_nz": bool(np.any(d["biasp"] != 0)),
        "bias2_nz": bool(np.any(d["bias2"] != 0)),
    }
    return d, flags


# ----------------------------------------------------------------------------
# device kernel
# ----------------------------------------------------------------------------

def build(flags):
    import concourse.bass as bass
    import concourse.tile as tile
    from concourse import bacc, mybir

    f32 = mybir.dt.float32
    bf16 = mybir.dt.bfloat16
    f8 = mybir.dt.float8e4
    AF = mybir.ActivationFunctionType
    OP = mybir.AluOpType
    DR = mybir.MatmulPerfMode.DoubleRow

    nc = bacc.Bacc("TRN2", target_bir_lowering=False)

    xd = nc.dram_tensor("x", [T, C], f32, kind="ExternalInput")
    wq8d = nc.dram_tensor("wq8", [128, NCI, C], f8, kind="ExternalInput")
    wk8d = nc.dram_tensor("wk8", [128, NCI, C], f8, kind="ExternalInput")
    wv8d = nc.dram_tensor("wv8", [128, NCI, C], f8, kind="ExternalInput")
    wp8d = nc.dram_tensor("wp8", [128, NCI, C], f8, kind="ExternalInput")
    w18d = nc.dram_tensor("w18", [128, NCI, FF], f8, kind="ExternalInput")
    w28d = nc.dram_tensor("w28", [128, NFF, C], f8, kind="ExternalInput")
    qbd = nc.dram_tensor("qb", [128, 8], f32, kind="ExternalInput")
    kbd = nc.dram_tensor("kb", [128, 8], f32, kind="ExternalInput")
    b1ed = nc.dram_tensor("b1e", [128, NFF], f32, kind="ExternalInput")
    masknd = nc.dram_tensor("maskneg", [128, 128], bf16, kind="ExternalInput")
    vbpd = nc.dram_tensor("vbpad", [H * VW], f32, kind="ExternalInput")
    biaspd = nc.dram_tensor("biasp", [C], f32, kind="ExternalInput")
    bias2d = nc.dram_tensor("bias2", [C], f32, kind="ExternalInput")
    outd = nc.dram_tensor("out", [T, C], f32, kind="ExternalOutput")

    def bcast_dram_row(vec_ap, n):
        return bass.AP(tensor=vec_ap.tensor, offset=vec_ap.offset,
                       ap=[[0, 128], [1, n]])

    def dma_chunked(dst, src, dim1, n=1):
        # n DMAs spread across queues (high n steals SBUF bandwidth from PE)
        step = dim1 // n
        for i in range(n):
            nc.sync.dma_start(dst[:, step * i:step * (i + 1), :],
                              src[:, step * i:step * (i + 1), :])

    with tile.TileContext(nc) as tc, \
         tc.tile_pool(name="consts", bufs=1) as consts, \
         tc.tile_pool(name="xpool", bufs=1) as xpool, \
         tc.tile_pool(name="hpool", bufs=1) as hpool, \
         tc.tile_pool(name="ln_tmp", bufs=3) as ln_tmp:

        # ---- constants ----
        from concourse.masks import make_identity
        ident8 = consts.tile([128, 128], f8, name="ident8")
        make_identity(nc, ident8[:])
        identb = consts.tile([128, 128], bf16, name="identb")
        make_identity(nc, identb[:])
        maskn_sb = consts.tile([128, 128], bf16, name="maskn_sb")
        nc.sync.dma_start(maskn_sb[:], masknd[:, :])
        eps_sb = consts.tile([128, 1], f32, name="eps_sb")
        nc.vector.memset(eps_sb[:], EPS)
        if flags["qb_nz"]:
            qb_sb = consts.tile([128, 8], f32, name="qb_sb")
            nc.sync.dma_start(qb_sb[:], qbd[:, :])
        if flags["kb_nz"]:
            kb_sb = consts.tile([128, 8], f32, name="kb_sb")
            nc.sync.dma_start(kb_sb[:], kbd[:, :])
        if flags["b1_nz"]:
            b1e_sb = consts.tile([128, NFF], f32, name="b1e_sb")
            nc.sync.dma_start(b1e_sb[:], b1ed[:, :])
        if flags["vb_nz"]:
            vb_sb = consts.tile([128, H * VW], f32, name="vb_sb")
            nc.sync.dma_start(vb_sb[:], bcast_dram_row(vbpd[:], H * VW))
        if flags["biasp_nz"]:
            bp_sb = consts.tile([128, C], f32, name="bp_sb")
            nc.sync.dma_start(bp_sb[:], bcast_dram_row(biaspd[:], C))
        if flags["bias2_nz"]:
            b2_sb = consts.tile([128, C], f32, name="b2_sb")
            nc.sync.dma_start(b2_sb[:], bcast_dram_row(bias2d[:], C))

        # ---- x tiles (persist; become xnew, then the output) ----
        x_sb = []
        for t in range(NT):
            xt = xpool.tile([128, C], f32, name=f"x{t}")
            for q in range(4):
                nc.sync.dma_start(xt[:, 256 * q:256 * (q + 1)],
                                  xd[128 * t:128 * (t + 1),
                                     256 * q:256 * (q + 1)])
            x_sb.append(xt)

        # hT in two t-halves so consumers can start after 4 t-tiles
        hT = [hpool.tile([128, NCI, 512], f8, name=f"hT{i}") for i in range(2)]

        def ln_tile(t):
            """LN stats + normalize for x tile t -> fp8 h tile (returns it)."""
            xt = x_sb[t]
            stats = ln_tmp.tile([128, 2, 6], f32, tag="lnstats")
            nc.vector.bn_stats(stats[:, 0, :], xt[:, 0:512])
            nc.vector.bn_stats(stats[:, 1, :], xt[:, 512:1024])
            mv = ln_tmp.tile([128, 2], f32, tag="lnmv")
            nc.vector.bn_aggr(mv[:], stats[:])
            rstd = ln_tmp.tile([128, 1], f32, tag="lnrstd")
            nc.scalar.activation(rstd[:], mv[:, 1:2], AF.Sqrt, bias=eps_sb[:])
            nc.vector.reciprocal(rstd[:], rstd[:])
            nmr = ln_tmp.tile([128, 1], f32, tag="lnnmr")
            nc.vector.tensor_scalar(nmr[:], mv[:, 0:1], rstd[:], -1.0,
                                    OP.mult, OP.mult)
            ht = ln_tmp.tile([128, C], f8, tag="lnh")
            nc.scalar.activation(ht[:], xt[:], AF.Identity,
                                 bias=nmr[:], scale=rstd[:])
            return ht

        def ln_transpose(t, ht, ps_tr):
            """PE-transpose h tile t into hT[t // 4] (batched evacuation)."""
            ptr = ps_tr.tile([128, NCI, 128, 2], f8, tag="tr")
            for c in range(NCI):
                nc.tensor.transpose(ptr[:, c, :, 0],
                                    ht[:, 128 * c:128 * (c + 1)], ident8[:])
            half, tl = divmod(t, 4)
            nc.scalar.activation(hT[half][:, :, 128 * tl:128 * (tl + 1)],
                                 ptr[:, :, :, 0], AF.Identity)

        with tc.tile_pool(name="wpp", bufs=1) as wpp, \
             tc.tile_pool(name="w1p", bufs=1) as w1p, \
             tc.tile_pool(name="att", bufs=1) as att, \
             tc.tile_pool(name="qkv", bufs=1) as qkv:
            attT8 = att.tile([128, NCI, T], f8, name="attT8")
            qT_sb = [qkv.tile([128, T], f8, name=f"qT{p}") for p in range(8)]
            kT_sb = [qkv.tile([128, T], f8, name=f"kT{p}") for p in range(8)]
            v8 = qkv.tile([128, NT, H, VW], f8, name="v8")
            # ones in cols 0..63 -> avp rows 0..63 all hold the softmax sums
            nc.gpsimd.memset(v8[:, :, :, 0:HS], 1.0)

            with tc.tile_pool(name="wqkv", bufs=1) as wqkv:
                wq8 = wqkv.tile([128, NCI, C], f8, name="wq8")
                dma_chunked(wq8, wq8d, NCI)
                wk8 = wqkv.tile([128, NCI, C], f8, name="wk8")
                dma_chunked(wk8, wk8d, NCI)
                wv8 = wqkv.tile([128, NCI, C], f8, name="wv8")
                dma_chunked(wv8, wv8d, NCI)

                # ====================== phase 1: LN1 ========================
                with tc.tile_pool(name="ps_tr1", bufs=2, space="PSUM") as ps_tr:
                    for t in range(NT):
                        ln_transpose(t, ln_tile(t), ps_tr)

                # ====================== phase 2: QKV ========================
                with tc.tile_pool(name="ps_qkv", bufs=2, space="PSUM") as ps_qkv:
                    for (w8, dst, bias_nm, b_nz) in (
                            (wq8, qT_sb, "qb", flags["qb_nz"]),
                            (wk8, kT_sb, "kb", flags["kb_nz"])):
                        for p in range(8):
                            ps = ps_qkv.tile([128, T], f32, tag="mm")
                            for half in (0, 1):
                                for cp in range(4):
                                    nc.tensor.matmul(
                                        ps[:, 512 * half:512 * half + 512],
                                        lhsT=w8[:, 2 * cp:2 * cp + 2,
                                                128 * p:128 * (p + 1)],
                                        rhs=hT[half][:, 2 * cp:2 * cp + 2, :],
                                        start=(cp == 0), stop=(cp == 3),
                                        perf_mode=DR)
                            if b_nz:
                                bias_sb = qb_sb if bias_nm == "qb" else kb_sb
                                nc.scalar.activation(dst[p][:], ps[:],
                                                     AF.Identity,
                                                     bias=bias_sb[:, p:p + 1])
                            else:
                                nc.scalar.activation(dst[p][:], ps[:],
                                                     AF.Identity)
                    # v: [t-part, head-major d]
                    for t in range(NT):
                        half, tl = divmod(t, 4)
                        ps = ps_qkv.tile([128, T], f32, tag="mm")
                        for off in (0, 512):
                            for cp in range(4):
                                nc.tensor.matmul(
                                    ps[:, off:off + 512],
                                    lhsT=hT[half][:, 2 * cp:2 * cp + 2,
                                                  128 * tl:128 * (tl + 1)],
                                    rhs=wv8[:, 2 * cp:2 * cp + 2,
                                            off:off + 512],
                                    start=(cp == 0), stop=(cp == 3),
                                    perf_mode=DR)
                        ps3 = ps[:].rearrange("p (h d) -> p h d", d=HS)
                        if flags["vb_nz"]:
                            vb3 = vb_sb[:].rearrange("p (h w) -> p h w", w=VW)
                            nc.vector.tensor_tensor(v8[:, t, :, HS:2 * HS],
                                                    ps3,
                                                    vb3[:, :, HS:2 * HS],
                                                    OP.add)
                        else:
                            nc.vector.tensor_copy(v8[:, t, :, HS:2 * HS], ps3)
            # wqkv pool closed - wq/wk/wv freed before attention

            # ===================== phase 3: attention =======================
            # prefetch Wp and W1 while attention runs
            wp8 = wpp.tile([128, NCI, C], f8, name="wp8")
            dma_chunked(wp8, wp8d, NCI)
            w18 = w1p.tile([128, NCI, FF], f8, name="w18")
            dma_chunked(w18, w18d, NCI)

            with tc.tile_pool(name="exp_pool", bufs=1) as exp_pool, \
                 tc.tile_pool(name="r_pool", bufs=2) as r_pool, \
                 tc.tile_pool(name="rr_pool", bufs=2) as rr_pool, \
                 tc.tile_pool(name="ps_st", bufs=2, space="PSUM") as ps_st, \
                 tc.tile_pool(name="ps_av", bufs=2, space="PSUM") as ps_av:
                ex2 = [exp_pool.tile([128, NT, 2, T], f8, name=f"exbuf{i}")
                       for i in range(2)]
                # zero-fill the non-causal 128-col blocks of odd s-tiles
                # (DoubleRow pairs s-tiles (2a, 2a+1); the shared window
                # starts at t=256a). exp never writes these regions, so one
                # fill per double-buffer suffices for all pairs.
                for i in range(2):
                    for a in range(4):
                        for e in (0, 1):
                            nc.gpsimd.memset(
                                ex2[i][:, 2 * a + 1, e,
                                       256 * a:256 * a + 128], 0.0)
                for p8 in range(8):
                    ex = ex2[p8 % 2]
                    for j in range(NT):
                        W = T - 128 * j
                        for e in (0, 1):
                            po = 64 * e
                            st = ps_st.tile([128, T], f32, tag="st",
                                            name=f"st{p8}_{j}_{e}")
                            for off in range(0, W, 512):
                                w = min(512, W - off)
                                nc.tensor.matmul(
                                    st[:, off:off + w],
                                    lhsT=kT_sb[p8][po:po + 64,
                                                   128 * j:128 * (j + 1)],
                                    rhs=qT_sb[p8][po:po + 64,
                                                  128 * j + off:
                                                  128 * j + off + w],
                                    start=True, stop=True)
                            nc.tensor.matmul(
                                st[:, 0:128], lhsT=identb[:],
                                rhs=maskn_sb[:], start=False, stop=True,
                                skip_group_check=True)
                            nc.scalar.activation(
                                ex[:, j, e, 128 * j:T], st[:, 0:W],
                                AF.Exp, scale=SM_SCALE)
                    for e in (0, 1):
                        h = 2 * p8 + e
                        avp_e = ps_av.tile([VW, T], f32, tag="av",
                                           name=f"av{h}")
                        for off in (0, 512):
                            aa = [q for q in range(4) if 256 * q < off + 512]
                            for a in aa:
                                lo = max(off, 256 * a)
                                nc.tensor.matmul(
                                    avp_e[0:VW, lo:off + 512],
                                    lhsT=v8[:, 2 * a:2 * a + 2, h, 0:VW],
                                    rhs=ex[:, 2 * a:2 * a + 2, e,
                                           lo:off + 512],
                                    start=(a == aa[0]), stop=(a == aa[-1]),
                                    perf_mode=DR)
                        # r = 1/sums (avp row 0); attT = att_unnorm * r
                        # (fp8). reciprocal_approx_fast mis-executes on
                        # 1-partition APs: recip the whole avp, use row 0.
                        if True:
                            r_sb = r_pool.tile([VW, T], f32, tag="r",
                                               name=f"r{2 * p8 + e}")
                            nc.vector.reciprocal_approx_fast(r_sb[:],
                                                             avp_e[0:VW, :])
                            rr = rr_pool.tile([128, T], f32, tag="rr",
                                              name=f"rr{2 * p8 + e}")
                            nc.gpsimd.partition_broadcast(rr[:], r_sb[0:1, :],
                                                          channels=128)
                            nc.vector.tensor_tensor(
                                attT8[64 * e:64 * e + 64, p8, :],
                                avp_e[HS:2 * HS, :], rr[0:64, :], OP.mult)
        # qkv + attention pools closed here

            # ============== phase 4+5: proj + LN2 (staggered) ===============
            with tc.tile_pool(name="w2p", bufs=1) as w2p, \
                 tc.tile_pool(name="f1pool", bufs=1) as f1pool:
                w28 = w2p.tile([128, NFF, C], f8, name="w28")
                dma_chunked(w28, w28d, NFF, 1)

                with tc.tile_pool(name="ps_proj", bufs=2, space="PSUM") as ps_proj, \
                     tc.tile_pool(name="ps_tr2", bufs=2, space="PSUM") as ps_tr:
                    pending = None  # (t, ht) awaiting PE transposes
                    for t in range(NT):
                        ps = ps_proj.tile([128, C], f32, tag="mm")
                        for off in (0, 512):
                            for cp in range(4):
                                nc.tensor.matmul(
                                    ps[:, off:off + 512],
                                    lhsT=attT8[:, 2 * cp:2 * cp + 2,
                                               128 * t:128 * (t + 1)],
                                    rhs=wp8[:, 2 * cp:2 * cp + 2,
                                            off:off + 512],
                                    start=(cp == 0), stop=(cp == 3),
                                    perf_mode=DR)
                        if pending is not None:
                            ln_transpose(pending[0], pending[1], ps_tr)
                        nc.vector.tensor_tensor(x_sb[t][:], ps[:], x_sb[t][:],
                                                OP.add)
                        if flags["biasp_nz"]:
                            nc.vector.tensor_tensor(x_sb[t][:], x_sb[t][:],
                                                    bp_sb[:], OP.add)
                        pending = (t, ln_tile(t))
                    ln_transpose(pending[0], pending[1], ps_tr)

                # ======================= phase 6: FFN =======================
                f1 = [f1pool.tile([128, NFF, 512], f8, name=f"f1_{i}")
                      for i in range(2)]
                with tc.tile_pool(name="ps_f1", bufs=2, space="PSUM") as ps_f1, \
                     tc.tile_pool(name="ps_y2", bufs=1, space="PSUM") as ps_y2:

                    def ffn1_half(half):
                        for f in range(NFF):
                            ps = ps_f1.tile([128, 512], f32, tag="mm")
                            for cp in range(4):
                                nc.tensor.matmul(
                                    ps[:],
                                    lhsT=w18[:, 2 * cp:2 * cp + 2,
                                             128 * f:128 * (f + 1)],
                                    rhs=hT[half][:, 2 * cp:2 * cp + 2, :],
                                    start=(cp == 0), stop=(cp == 3),
                                    perf_mode=DR)
                            if flags["b1_nz"]:
                                nc.scalar.activation(f1[half][:, f, :], ps[:],
                                                     AF.Relu,
                                                     bias=b1e_sb[:, f:f + 1])
                            elif f % 2 == 0:
                                nc.scalar.activation(f1[half][:, f, :], ps[:],
                                                     AF.Relu)
                            else:
                                nc.vector.tensor_scalar_max(f1[half][:, f, :],
                                                            ps[:], 0.0)

                    def ffn2_group(tg):
                        for off in (0, 512):
                            trange = range(4 * tg, 4 * tg + 4)
                            y2 = {t: ps_y2.tile([128, 512], f32,
                                                tag=f"y2_{t % 4}",
                                                name=f"y2_{off}_{t}")
                                  for t in trange}
                            for fp in range(NFF // 2):
                                for t in trange:
                                    tl = t % 4
                                    nc.tensor.matmul(
                                        y2[t][:],
                                        lhsT=f1[tg][:, 2 * fp:2 * fp + 2,
                                                    128 * tl:128 * (tl + 1)],
                                        rhs=w28[:, 2 * fp:2 * fp + 2,
                                                off:off + 512],
                                        start=(fp == 0), stop=(fp == 15),
                                        perf_mode=DR)
                            for t in trange:
                                nc.vector.tensor_tensor(
                                    x_sb[t][:, off:off + 512], y2[t][:],
                                    x_sb[t][:, off:off + 512], OP.add)
                                if flags["bias2_nz"]:
                                    nc.vector.tensor_tensor(
                                        x_sb[t][:, off:off + 512],
                                        x_sb[t][:, off:off + 512],
                                        b2_sb[:, off:off + 512], OP.add)
                                if off == 512:
                                    nc.sync.dma_start(
                                        outd[128 * t:128 * (t + 1), 0:512],
                                        x_sb[t][:, 0:512])
                                    nc.sync.dma_start(
                                        outd[128 * t:128 * (t + 1), 512:1024],
                                        x_sb[t][:, 512:1024])

                    ffn1_half(0)   # f1[0] = all f for t 0..511
                    ffn2_group(0)  # FFN2 on t 0..511 chases FFN1 half 1
                    ffn1_half(1)
                    ffn2_group(1)
    nc.compile()
    return nc


def _get_nc(flags):
    key = tuple(sorted(flags.items()))
    if key not in _CACHE:
        _CACHE[key] = build(flags)
    return _CACHE[key]


# ----------------------------------------------------------------------------
# public entry point
# ----------------------------------------------------------------------------

def kernel(**inputs):
    from concourse import bass_utils
    x = np.asarray(inputs["x"], np.float32)
    d, flags = _prep(inputs)
    nc = _get_nc(flags)
    in_maps = []
    for b in range(B):
        m = dict(d)
        m["x"] = np.ascontiguousarray(x[b])
        in_maps.append(m)
    res = bass_utils.run_bass_kernel_spmd(nc, in_maps, core_ids=list(range(B)))
    out = np.stack([r["out"] for r in res.results]).astype(np.float32)
    return out
